# revision 64
# baseline (speedup 1.0000x reference)
"""Graphormer layer (LocalSubgraphEncoder) Trainium2 Bass kernel, v2.

Sharding: node-parallel over 8 cores. Core i computes the full layer output
for query nodes [512*i, 512*i+512): all 8 heads of attention over all 4096
key nodes, edge-type bias, softmax, output projection, residual, LayerNorm.
No cross-core communication; host concatenates row slices.

v2 design (from perfetto analysis of v1: PE saturated by unpacked K=32
matmuls, GPSIMD dense local_scatter, STT stuck in 1x mode):
 - all matmuls bf16; 2-head row-packing for QK (tile_position row groups)
   and 2-head column-packing for PV / denominator matmuls.
 - scores layout S^T [keys(part), queries(free)]: softmax denominator z
   comes from a packed ones-vector matmul into a shared PSUM bank.
 - edge bias applied multiplicatively AFTER exp: P = exp(S) * F where
   F = exp(scattered bias) is precomputed DENSE on the host and streamed
   from HBM (33.5 MB/core) -> one 2x-mode DVE tensor_tensor per tile;
   GPSIMD does nothing.
 - ACT (ScalarE) does exclusively the exp drain PSUM->SBUF bf16 in
   [128,1024] tiles: the ~128 us floor every design shares.
 - biases fused into DVE copies (per-partition scalar AP) or rank-1 PE
   matmuls; LayerNorm scale/shift via outer-product matmuls.
"""
import os
import sys
import math
import numpy as np

sys.path.insert(0, "/opt/trn_rl_repo")
import ml_dtypes  # noqa: E402
from concourse import bacc, bass, mybir, tile  # noqa: E402
from concourse.bass_utils import run_bass_kernel_spmd  # noqa: E402

N, D, H, E, NT = 4096, 256, 8, 131072, 16
DH = D // H            # 32
NCORES = 8
Q = N // NCORES        # 512 query nodes per core
KB = 128               # key-node block (partition dim)
NKB = N // KB          # 32
NPAIR = NKB // 2       # 16 (two key-blocks per [128,1024] score tile)
LN_EPS = 1e-5
SCALE = 1.0 / math.sqrt(DH)

f32 = mybir.dt.float32
bf16 = mybir.dt.bfloat16
EXP = mybir.ActivationFunctionType.Exp
LN = mybir.ActivationFunctionType.Ln
ADD = mybir.AluOpType.add
MULT = mybir.AluOpType.mult
SUB = mybir.AluOpType.subtract

_PROG = {}
LAST_RESULTS = None

WARM_START = int(os.environ.get("WARM_START", "0"))
WARM_BOOST = int(os.environ.get("WARM_BOOST", "0"))
WARM_PERIOD = int(os.environ.get("WARM_PERIOD", "0"))
WARM_LEN = int(os.environ.get("WARM_LEN", "8"))


def build_program(skip_bias=False):
    nc = bacc.Bacc(
        "TRN2", target_bir_lowering=False, debug=False, num_devices=NCORES
    )

    def din(name, shape, dt):
        return nc.dram_tensor(name, shape, dt, kind="ExternalInput").ap()

    # (x + pos)^T stored as (half, slab, part, 512) so each chunk is a
    # contiguous 128 KiB transfer and projections start on the first chunk
    hT_d = din("hT", [2 * 8 * KB, Q], bf16)
    xqT_d = din("xqT", [D, Q], f32)         # x^T core slice (residual)
    Wq_d = din("Wq", [D, D], bf16)
    Wk_d = din("Wk", [D, D], bf16)
    Wv_d = din("Wv", [D, D], bf16)
    Wo_d = din("Wo", [D, D], bf16)
    bq_d = din("bq", [D, 1], f32)
    bk_d = din("bk", [D, 1], f32)
    bo_d = din("bo", [D, 1], f32)
    bv_d = din("bv", [1, D], bf16)
    gm_d = din("gm", [1, D], bf16)          # gamma row
    bt_d = din("bt", [1, D], bf16)          # beta row
    e128_d = din("e128", [KB, KB], bf16)    # block-broadcast matrix
    # dense exp(bias): row = (mh, t, partition), col = (h4, j, q)
    F_d = din("F", [2 * NPAIR * KB, 4 * 2 * Q], bf16)
    outT = nc.dram_tensor("outT", [D, Q], f32, kind="ExternalOutput").ap()

    hqT_d = din("hqT", [D, Q], bf16)        # h^T core query slice

    with tile.TileContext(nc) as tc:
        from contextlib import ExitStack

        with ExitStack() as ctx:
            cpool = ctx.enter_context(tc.tile_pool(name="consts", bufs=1))

            def ctile(shape, dt, tag):
                return cpool.tile(shape, dt, tag=tag, name=tag)

            # persistent SBUF residents
            hT = [ctile([KB, N], bf16, f"hT{c}") for c in range(2)]
            hq = [ctile([KB, Q], bf16, f"hq{c}") for c in range(2)]
            xq = [ctile([KB, Q], f32, f"xq{c}") for c in range(2)]
            wq = [ctile([KB, D], bf16, f"wq{c}") for c in range(2)]
            wk = [ctile([KB, D], bf16, f"wk{c}") for c in range(2)]
            wv = [ctile([KB, D], bf16, f"wv{c}") for c in range(2)]
            wo = [ctile([KB, D], bf16, f"wo{c}") for c in range(2)]
            bq = [ctile([KB, 1], f32, f"bq{c}") for c in range(2)]
            bk = [ctile([KB, 1], f32, f"bk{c}") for c in range(2)]
            bo = [ctile([KB, 1], f32, f"bo{c}") for c in range(2)]
            bv_r = ctile([1, D], bf16, "bv_r")
            gm = ctile([1, D], bf16, "gm")
            bt = ctile([1, D], bf16, "bt")
            e128 = ctile([KB, KB], bf16, "e128")
            kT = [ctile([KB, N], bf16, f"kT{c}") for c in range(2)]
            qTb = [ctile([KB, Q], bf16, f"qTb{c}") for c in range(2)]
            # V with ones column: [key, kb, h, 32 dims + 1 one]
            vSB = ctile([KB, NKB, H, DH + 1], bf16, "vSB")
            attnT = [ctile([KB, Q], bf16, f"attnT{c}") for c in range(2)]
            ones_1x128 = ctile([1, KB], bf16, "o1x128")
            ones_128x1 = ctile([KB, 1], bf16, "o128x1")
            ones_1xQ = ctile([1, Q], bf16, "o1xQ")
            epsT = ctile([1, 1], f32, "epsT")
            zer_1xQ = ctile([1, Q], f32, "z1xQ")

            # ---- loads ----
            # DMA priority: Q-proj inputs first, then per-slab hT chunks
            # (both halves per slab so K-proj slab s starts ASAP)
            for c in range(2):
                sl = slice(c * KB, (c + 1) * KB)
                nc.sync.dma_start(out=wq[c][:], in_=Wq_d[sl, :])
                nc.sync.dma_start(out=hq[c][:], in_=hqT_d[sl, :])
                nc.sync.dma_start(out=wk[c][:], in_=Wk_d[sl, :])
                nc.sync.dma_start(out=wv[c][:], in_=Wv_d[sl, :])
                nc.sync.dma_start(out=bq[c][:], in_=bq_d[sl, :])
                nc.sync.dma_start(out=bk[c][:], in_=bk_d[sl, :])
            for s in range(8):
                for c in range(2):
                    row = (c * 8 + s) * KB
                    nc.sync.dma_start(
                        out=hT[c][:, s * Q:(s + 1) * Q],
                        in_=hT_d[row:row + KB, :],
                    )
            nc.sync.dma_start(out=bv_r[:], in_=bv_d[:])
            nc.sync.dma_start(out=e128[:], in_=e128_d[:])
            for c in range(2):
                sl = slice(c * KB, (c + 1) * KB)
                nc.sync.dma_start(out=wo[c][:], in_=Wo_d[sl, :])
                nc.sync.dma_start(out=xq[c][:], in_=xqT_d[sl, :])
                nc.sync.dma_start(out=bo[c][:], in_=bo_d[sl, :])
            nc.sync.dma_start(out=gm[:], in_=gm_d[:])
            nc.sync.dma_start(out=bt[:], in_=bt_d[:])
            nc.vector.memset(ones_1x128[:], 1.0)
            nc.vector.memset(ones_128x1[:], 1.0)
            nc.vector.memset(epsT[:], LN_EPS)
            nc.vector.memset(zer_1xQ[:], 0.0)

            # preload the exp ACT table during projections so the first real
            # exp doesn't stall the attention pipeline for ~2.7us; the output
            # is live (ones_1xQ = exp(0)) so DCE keeps it.
            nc.scalar.activation(ones_1xQ[:], zer_1xQ[:], EXP)

            # ---- projections (all bf16, biases fused) ----
            with tc.tile_pool(name="pps", bufs=3, space="PSUM") as pps:
                IDENT = mybir.ActivationFunctionType.Identity
                # Q^T [2][128, 512] head-major partitions; bias fused into
                # the ACT copy (per-partition bias is native there)
                for mh in range(2):
                    ps = pps.tile([KB, Q], f32, tag="proj", name="proj")
                    for kc in range(2):
                        nc.tensor.matmul(
                            ps[:], wq[kc][:, mh * KB:(mh + 1) * KB], hq[kc][:],
                            start=(kc == 0), stop=(kc == 1),
                        )
                    nc.scalar.activation(
                        qTb[mh][:], ps[:], IDENT, bias=bq[mh][:]
                    )
                # K^T [2][128, 4096]  (interleave mh so attention t=0 for
                # both halves unblocks early)
                for s in range(8):
                    for mh in range(2):
                        ssl = slice(s * Q, (s + 1) * Q)
                        ps = pps.tile([KB, Q], f32, tag="proj", name="proj")
                        for kc in range(2):
                            nc.tensor.matmul(
                                ps[:], wk[kc][:, mh * KB:(mh + 1) * KB],
                                hT[kc][:, ssl],
                                start=(kc == 0), stop=(kc == 1),
                            )
                        nc.scalar.activation(
                            kT[mh][:, ssl], ps[:], IDENT, bias=bk[mh][:]
                        )
                # V node-major [128, kb, h, 33] + bias via rank-1
                nc.vector.memset(vSB[:, :, :, DH], 1.0)
                for kb_i in range(NKB):
                    ksl = slice(kb_i * KB, (kb_i + 1) * KB)
                    psv = pps.tile([KB, D], f32, tag="projv", name="projv")
                    for kc in range(2):
                        nc.tensor.matmul(
                            psv[:], hT[kc][:, ksl], wv[kc][:],
                            start=(kc == 0),
                            stop=(kc == 1 and skip_bias),
                        )
                    if not skip_bias:
                        nc.tensor.matmul(
                            psv[:], ones_1x128[:], bv_r[:],
                            start=False, stop=True,
                        )
                    nc.vector.tensor_copy(vSB[:, kb_i, :, 0:DH], psv[:])

            # ---- attention ----
            with ExitStack() as actx:
                sps = actx.enter_context(
                    tc.tile_pool(name="sps", bufs=3, space="PSUM")
                )
                ops = actx.enter_context(
                    tc.tile_pool(name="ops", bufs=2, space="PSUM")
                )
                spool = actx.enter_context(tc.tile_pool(name="spool", bufs=6))
                fpool = actx.enter_context(tc.tile_pool(name="fpool", bufs=3))
                npool = actx.enter_context(tc.tile_pool(name="npool", bufs=2))

                norm_tail = []
                for mh in range(2):
                    # one oacc tile per head pair: partitions 0:33 head A
                    # (32 dims + z), 64:97 head B.
                    oacc = [
                        ops.tile([KB, Q], f32, tag="oacc", name="oacc")
                        for _ in range(2)
                    ]

                    def issue_pv(g):
                        t, pr, pf = g
                        first = (t == 0)
                        last = (t == NPAIR - 1)
                        for j in range(2):
                            kb_i = 2 * t + j
                            for hp in range(2):
                                h4 = 2 * pr + hp
                                h = 4 * mh + h4
                                nc.tensor.matmul(
                                    oacc[pr][64 * hp:64 * hp + DH + 1, :],
                                    vSB[:, kb_i, h, :],
                                    pf[j][:, hp * Q:(hp + 1) * Q],
                                    start=(first and j == 0),
                                    stop=(last and j == 1),
                                    tile_position=(0, 64 * hp),
                                    skip_group_check=True,
                                )

                    # HAM warm-up: dependency-free burst of matmuls into the
                    # oacc region; the first real PV starts with start=True
                    # so the garbage is overwritten.
                    for w in range(WARM_START):
                        nc.tensor.matmul(
                            oacc[0][0:DH + 1, :], vSB[:, 0, 0, :], qTb[mh][:],
                            start=True, stop=True,
                            tile_position=(0, 0), skip_group_check=True,
                        )
                    # software pipeline, lag 2: PV(g-2) issues BEFORE QK(g)
                    # so the in-order PE always has dependency-free work.
                    from collections import deque
                    pend = deque()
                    for t in range(NPAIR):
                        if WARM_PERIOD and mh == 0 and t == WARM_PERIOD:
                            # periodic dense matmul burst to re-flip HAM
                            bt_ps = sps.tile([KB, 2 * Q], f32, tag="sg",
                                             name="warm")
                            for w in range(WARM_LEN):
                                nc.tensor.matmul(
                                    bt_ps[:, 0:Q],
                                    kT[mh][0:32, 0:KB], qTb[mh][0:32, :],
                                    start=True, stop=True,
                                    tile_position=(0, 0),
                                    skip_group_check=True,
                                )
                        # one 1 MiB F transfer covers (mh, t) x 4 heads
                        fbig = fpool.tile([KB, 8 * Q], bf16, tag="ft",
                                          name="ft")
                        row = (mh * NPAIR + t) * KB
                        nc.sync.dma_start(
                            out=fbig[:], in_=F_d[row:row + KB, :]
                        )
                        for pr in range(2):      # head pairs (2p, 2p+1)
                            if len(pend) >= 2:
                                issue_pv(pend.popleft())
                            # score tiles pair TWO HEADS at the same j so the
                            # two QK matmuls of a tile land on different PE
                            # row groups and run concurrently (row packing).
                            sg = [
                                sps.tile([KB, 2 * Q], f32, tag="sg", name="sg")
                                for _ in range(2)
                            ]
                            # density boosters (optional): dummy weight loads
                            for w in range(WARM_BOOST):
                                nc.tensor.ldweights(
                                    kT[mh][:, 0:KB], tile_position=(0, 0),
                                )
                            for j in range(2):
                                kb_i = 2 * t + j
                                ksl = slice(kb_i * KB, (kb_i + 1) * KB)
                                for hp in range(2):
                                    h4 = 2 * pr + hp
                                    psl = slice(32 * h4, 32 * h4 + 32)
                                    nc.tensor.matmul(
                                        sg[j][:, hp * Q:(hp + 1) * Q],
                                        kT[mh][psl, ksl],
                                        qTb[mh][psl, :],
                                        start=True, stop=True,
                                        tile_position=(32 * h4, 0),
                                    )
                            pf = [None, None]
                            for j in range(2):
                                # exp (ACT) PSUM -> SBUF bf16
                                p0 = spool.tile(
                                    [KB, 2 * Q], bf16, tag="p0", name="p0"
                                )
                                nc.scalar.activation(
                                    p0[:], sg[j][:], EXP, scale=SCALE
                                )
                                pf[j] = spool.tile(
                                    [KB, 2 * Q], bf16, tag="pf", name="pf"
                                )
                                nc.vector.tensor_mul(
                                    pf[j][:], p0[:],
                                    fbig[:, (pr * 2 + j) * 2 * Q:
                                         (pr * 2 + j + 1) * 2 * Q],
                                )
                            pend.append((t, pr, pf))
                    while pend:
                        issue_pv(pend.popleft())

                    # ---- normalize, DVE part: compact numerators + 1/z ----
                    # (the PE-dependent broadcast matmul is deferred for mh0
                    # so it doesn't block mh1's QKs in the in-order PE queue)
                    if mh == 0:
                        # copy to SBUF, releasing oacc PSUM for mh1's PV
                        oaccS = [
                            npool.tile([KB, Q], f32, tag=f"oaccS{pr}",
                                       name=f"oaccS{pr}")
                            for pr in range(2)
                        ]
                        for pr in range(2):
                            nc.vector.tensor_copy(oaccS[pr][:], oacc[pr][:])
                        src = oaccS
                    else:
                        src = oacc  # tail: read PSUM directly
                    onum = npool.tile([KB, Q], f32, tag=f"onum{mh}",
                                      name=f"onum{mh}")
                    zsb = npool.tile([KB, Q], f32, tag="zsb", name="zsb")
                    nc.vector.memset(zsb[:], 1.0)
                    for h4 in range(4):
                        pr, hp = h4 >> 1, h4 & 1
                        nc.vector.tensor_copy(
                            onum[32 * h4:32 * h4 + 32, :],
                            src[pr][64 * hp:64 * hp + 32, :],
                        )
                        nc.vector.tensor_copy(
                            zsb[32 * h4:32 * h4 + 1, :],
                            src[pr][64 * hp + 32:64 * hp + 33, :],
                        )
                    rzb = npool.tile([KB, Q], bf16, tag=f"rzb{mh}",
                                     name=f"rzb{mh}")
                    if mh == 0:
                        # DVE reciprocal (ACT is saturated by the exp stream)
                        rz = npool.tile([KB, Q], f32, tag="rz", name="rz")
                        nc.vector.reciprocal_approx_fast(rz[:], zsb[:])
                        nc.vector.tensor_copy(rzb[:], rz[:])
                    else:
                        # tail: 1/z = exp(-ln z) on the now-idle ACT
                        lnz = npool.tile([KB, Q], f32, tag="lnz", name="lnz")
                        nc.scalar.activation(lnz[:], zsb[:], LN)
                        nc.scalar.activation(rzb[:], lnz[:], EXP, scale=-1.0)
                    norm_tail.append((mh, onum, rzb))

                # ---- normalize, PE part (after all attention matmuls) ----
                for mh, onum, rzb in norm_tail:
                    zbp = sps.tile([KB, Q], f32, tag="sg", name="zbp")
                    nc.tensor.matmul(
                        zbp[:], e128[:], rzb[:], start=True, stop=True
                    )
                    # mixed-space TT: exempt from equal-base-partition rule
                    nc.vector.tensor_mul(attnT[mh][:], onum[:], zbp[:])

            # ---- output projection + residual + LayerNorm ----
            with ExitStack() as ectx:
                rps = ectx.enter_context(
                    tc.tile_pool(name="rps", bufs=1, space="PSUM")
                )
                epool = ectx.enter_context(tc.tile_pool(name="epool", bufs=2))
                out2 = [
                    epool.tile([KB, Q], f32, tag=f"out2_{c}", name=f"out2_{c}")
                    for c in range(2)
                ]
                for c in range(2):
                    op_ps = rps.tile([KB, Q], f32, tag="oproj", name="oproj")
                    for mh in range(2):
                        nc.tensor.matmul(
                            op_ps[:],
                            wo[mh][:, c * KB:(c + 1) * KB],
                            attnT[mh][:],
                            start=(mh == 0), stop=(mh == 1),
                        )
                    # out2 = (psum + bo) + x
                    nc.vector.scalar_tensor_tensor(
                        out2[c][:], op_ps[:], bo[c][:], xq[c][:],
                        op0=ADD, op1=ADD,
                    )
                # stats: mu, s2 via ones matmuls (f32)
                ones_f = epool.tile([KB, 1], f32, tag="onesf", name="onesf")
                nc.vector.memset(ones_f[:], 1.0)
                mu_ps = rps.tile([1, Q], f32, tag="mu", name="mu")
                for c in range(2):
                    nc.tensor.matmul(
                        mu_ps[:], ones_f[:], out2[c][:],
                        start=(c == 0), stop=(c == 1),
                        skip_group_check=True,
                    )
                ones_b = epool.tile([KB, 1], bf16, tag="onesb", name="onesb")
                nc.vector.memset(ones_b[:], 1.0)
                s2_ps = rps.tile([1, Q], f32, tag="s2", name="s2")
                for c in range(2):
                    sq = epool.tile([KB, Q], bf16, tag="sq", name="sq")
                    nc.vector.tensor_mul(sq[:], out2[c][:], out2[c][:])
                    nc.tensor.matmul(
                        s2_ps[:], ones_b[:], sq[:],
                        start=(c == 0), stop=(c == 1),
                        skip_group_check=True,
                    )
                mu = epool.tile([1, Q], f32, tag="mu_s", name="mu_s")
                nc.vector.tensor_scalar_mul(mu[:], mu_ps[:], 1.0 / D)
                m2 = epool.tile([1, Q], f32, tag="m2", name="m2")
                nc.vector.tensor_mul(m2[:], mu[:], mu[:])
                var = epool.tile([1, Q], f32, tag="var", name="var")
                nc.vector.scalar_tensor_tensor(
                    var[:], s2_ps[:], 1.0 / D, m2[:], op0=MULT, op1=SUB,
                )
                # rstd = exp(-0.5*ln(var+eps)): stays in the exp/ln ACT
                # table set, avoiding a ~3us sqrt table switch
                lv = epool.tile([1, Q], f32, tag="lv", name="lv")
                nc.scalar.activation(lv[:], var[:], LN, bias=epsT[:])
                rstd = epool.tile([1, Q], f32, tag="rstd", name="rstd")
                nc.scalar.activation(rstd[:], lv[:], EXP, scale=-0.5)
                # broadcast tiles via outer products:
                # c1 = gamma (x) rstd ; c2 = beta (x) 1 - gamma (x) (mu*rstd)
                rstd_b = epool.tile([1, Q], bf16, tag="rstdb", name="rstdb")
                nc.vector.tensor_copy(rstd_b[:], rstd[:])
                mr = epool.tile([1, Q], f32, tag="mr", name="mr")
                nc.vector.tensor_mul(mr[:], mu[:], rstd[:])
                mrn = epool.tile([1, Q], bf16, tag="mrn", name="mrn")
                nc.vector.tensor_scalar_mul(mrn[:], mr[:], -1.0)
                for c in range(2):
                    csl = slice(c * KB, (c + 1) * KB)
                    c1p = rps.tile([KB, Q], f32, tag="c1", name="c1")
                    nc.tensor.matmul(
                        c1p[:], gm[:, csl], rstd_b[:], start=True, stop=True
                    )
                    # c2 = gamma (x) (-mu*rstd) + beta (x) 1
                    c2p = rps.tile([KB, Q], f32, tag="c2", name="c2")
                    nc.tensor.matmul(
                        c2p[:], gm[:, csl], mrn[:], start=True, stop=False
                    )
                    nc.tensor.matmul(
                        c2p[:], bt[:, csl], ones_1xQ[:],
                        start=False, stop=True,
                    )
                    t1 = epool.tile([KB, Q], f32, tag="t1", name="t1")
                    nc.vector.tensor_mul(t1[:], out2[c][:], c1p[:])
                    y = epool.tile([KB, Q], f32, tag="y", name="y")
                    nc.vector.tensor_add(y[:], t1[:], c2p[:])
                    nc.sync.dma_start(out=outT[csl, :], in_=y[:])

    nc.compile()
    return nc


def _prep_F(q_idx, k_idx, bias_eh):
    """Dense multiplicative bias F = exp(scattered bias), per core.

    Row-block order matches kernel consumption: [mh, t, h4, partition]."""
    key = q_idx.astype(np.int64) * N + k_idx.astype(np.int64)
    uk, inv = np.unique(key, return_inverse=True)
    acc = np.zeros((len(uk), H), np.float32)
    np.add.at(acc, inv, bias_eh)
    uq = (uk // N).astype(np.int32)
    ukey = (uk % N).astype(np.int32)
    vals16 = np.exp(acc).astype(ml_dtypes.bfloat16).view(np.uint16)

    Fs = []
    for i in range(NCORES):
        sel = (uq >> 9) == i
        q = uq[sel] & (Q - 1)
        k = ukey[sel]
        v = vals16[sel]
        t = k >> 8
        j = (k >> 7) & 1
        p = k & (KB - 1)
        # cols ordered (pr, j, hp, q) to match paired score tiles
        F16 = np.full((2, NPAIR, KB, 2, 2, 2, Q), 0x3F80, np.uint16)
        for h in range(H):
            F16[h >> 2, t, p, (h & 3) >> 1, j, h & 1, q] = v[:, h]
        Fs.append(
            np.ascontiguousarray(F16.reshape(2 * NPAIR * KB, 4 * 2 * Q))
            .view(ml_dtypes.bfloat16)
        )
    return Fs


def kernel(**inputs):
    global LAST_RESULTS, _PROG
    x = np.asarray(inputs["x"], np.float32)
    pos = np.asarray(inputs["pos_encoding"], np.float32)
    ei = np.asarray(inputs["edge_index"])
    et = np.asarray(inputs["edge_types"])
    emb = np.asarray(inputs["edge_emb"], np.float32)
    W = {k: np.asarray(inputs[k], np.float32) for k in ("Wq", "Wk", "Wv", "Wo")}
    b = {k: np.asarray(inputs[k], np.float32).reshape(-1)
         for k in ("bq", "bk", "bv", "bo", "gamma", "beta")}

    bias_eh = emb[et]  # [E, H]
    Fs = _prep_F(ei[0], ei[1], bias_eh)

    skip_bias = bool(np.all(b["bv"] == 0.0))
    pkey = (WARM_START, WARM_BOOST, WARM_PERIOD, WARM_LEN, skip_bias)
    if pkey not in _PROG:
        _PROG[pkey] = build_program(skip_bias=skip_bias)
    nc = _PROG[pkey]

    h = (x + pos).astype(np.float32)
    hT = np.ascontiguousarray(h.T.astype(ml_dtypes.bfloat16))
    # chunked layout (half, slab, part, 512)
    hTc = np.ascontiguousarray(
        hT.reshape(2, KB, 8, Q).transpose(0, 2, 1, 3).reshape(2 * 8 * KB, Q)
    )
    xT = np.ascontiguousarray(x.T)
    Wb = {k: np.ascontiguousarray(w.astype(ml_dtypes.bfloat16))
          for k, w in W.items()}
    col = lambda a: np.ascontiguousarray(a.reshape(D, 1))
    row16 = lambda a: np.ascontiguousarray(
        a.reshape(1, D).astype(ml_dtypes.bfloat16)
    )
    e128 = np.zeros((KB, KB), np.float32)
    for h4 in range(4):
        e128[32 * h4, 32 * h4:32 * h4 + 32] = 1.0
    e128 = np.ascontiguousarray(e128.astype(ml_dtypes.bfloat16))

    in_maps = []
    for i in range(NCORES):
        sl = slice(i * Q, (i + 1) * Q)
        in_maps.append({
            "hT": hTc,
            "hqT": np.ascontiguousarray(hT[:, sl]),
            "xqT": np.ascontiguousarray(xT[:, sl]),
            "Wq": Wb["Wq"], "Wk": Wb["Wk"], "Wv": Wb["Wv"], "Wo": Wb["Wo"],
            "bq": col(b["bq"]), "bk": col(b["bk"]), "bo": col(b["bo"]),
            "bv": row16(b["bv"]), "gm": row16(b["gamma"]),
            "bt": row16(b["beta"]), "e128": e128,
            "F": Fs[i],
        })

    trace = os.environ.get("BASS_KERNEL_TRACE", "0") == "1"
    try:
        res = run_bass_kernel_spmd(
            nc, in_maps, list(range(NCORES)), trace=trace
        )
    except Exception:
        if not trace:
            raise
        res = run_bass_kernel_spmd(nc, in_maps, list(range(NCORES)))
    LAST_RESULTS = res

    out = np.empty((N, D), np.float32)
    for i in range(NCORES):
        out[i * Q:(i + 1) * Q, :] = np.asarray(
            res.results[i]["outT"], np.float32
        ).T
    return out


# revision 65
# speedup vs baseline: 1.0165x; 1.0165x over previous
"""Graphormer layer (LocalSubgraphEncoder) Trainium2 Bass kernel, v2.

Sharding: node-parallel over 8 cores. Core i computes the full layer output
for query nodes [512*i, 512*i+512): all 8 heads of attention over all 4096
key nodes, edge-type bias, softmax, output projection, residual, LayerNorm.
No cross-core communication; host concatenates row slices.

v2 design (from perfetto analysis of v1: PE saturated by unpacked K=32
matmuls, GPSIMD dense local_scatter, STT stuck in 1x mode):
 - all matmuls bf16; 2-head row-packing for QK (tile_position row groups)
   and 2-head column-packing for PV / denominator matmuls.
 - scores layout S^T [keys(part), queries(free)]: softmax denominator z
   comes from a packed ones-vector matmul into a shared PSUM bank.
 - edge bias applied multiplicatively AFTER exp: P = exp(S) * F where
   F = exp(scattered bias) is precomputed DENSE on the host and streamed
   from HBM (33.5 MB/core) -> one 2x-mode DVE tensor_tensor per tile;
   GPSIMD does nothing.
 - ACT (ScalarE) does exclusively the exp drain PSUM->SBUF bf16 in
   [128,1024] tiles: the ~128 us floor every design shares.
 - biases fused into DVE copies (per-partition scalar AP) or rank-1 PE
   matmuls; LayerNorm scale/shift via outer-product matmuls.
"""
import os
import sys
import math
import numpy as np

sys.path.insert(0, "/opt/trn_rl_repo")
import ml_dtypes  # noqa: E402
from concourse import bacc, bass, mybir, tile  # noqa: E402
from concourse.bass_utils import run_bass_kernel_spmd  # noqa: E402

N, D, H, E, NT = 4096, 256, 8, 131072, 16
DH = D // H            # 32
NCORES = 8
Q = N // NCORES        # 512 query nodes per core
KB = 128               # key-node block (partition dim)
NKB = N // KB          # 32
NPAIR = NKB // 2       # 16 (two key-blocks per [128,1024] score tile)
LN_EPS = 1e-5
SCALE = 1.0 / math.sqrt(DH)

f32 = mybir.dt.float32
bf16 = mybir.dt.bfloat16
EXP = mybir.ActivationFunctionType.Exp
LN = mybir.ActivationFunctionType.Ln
ADD = mybir.AluOpType.add
MULT = mybir.AluOpType.mult
SUB = mybir.AluOpType.subtract

_PROG = {}
LAST_RESULTS = None

WARM_START = int(os.environ.get("WARM_START", "0"))
WARM_BOOST = int(os.environ.get("WARM_BOOST", "0"))
WARM_PERIOD = int(os.environ.get("WARM_PERIOD", "0"))
WARM_LEN = int(os.environ.get("WARM_LEN", "8"))


_TABLES_PATCHED = False


def _patch_act_tables():
    """Restrict the ACT table chooser to natural_log_exp_and_others (has
    exp, ln and identity) so the whole kernel needs ONE table load instead
    of bouncing between the exp and ln sets. Keys/order preserved so
    act_func_set_id indices stay valid."""
    global _TABLES_PATCHED
    if _TABLES_PATCHED:
        return
    from concourse import hw_specs
    import concourse.bacc as bacc_mod

    orig = hw_specs.get_activation_tables

    def patched(arch):
        t = orig(arch)
        keep = "natural_log_exp_and_others"
        if keep not in t:
            return t
        return {k: (v if k == keep else set()) for k, v in t.items()}

    bacc_mod.get_activation_tables = patched
    _TABLES_PATCHED = True


def build_program(skip_bias=False):
    _patch_act_tables()
    nc = bacc.Bacc(
        "TRN2", target_bir_lowering=False, debug=False, num_devices=NCORES
    )

    def din(name, shape, dt):
        return nc.dram_tensor(name, shape, dt, kind="ExternalInput").ap()

    # (x + pos)^T stored as (half, slab, part, 512) so each chunk is a
    # contiguous 128 KiB transfer and projections start on the first chunk
    hT_d = din("hT", [2 * 8 * KB, Q], bf16)
    xqT_d = din("xqT", [D, Q], f32)         # x^T core slice (residual)
    Wq_d = din("Wq", [D, D], bf16)
    Wk_d = din("Wk", [D, D], bf16)
    Wv_d = din("Wv", [D, D], bf16)
    Wo_d = din("Wo", [D, D], bf16)
    bq_d = din("bq", [D, 1], f32)
    bk_d = din("bk", [D, 1], f32)
    bo_d = din("bo", [D, 1], f32)
    bv_d = din("bv", [1, D], bf16)
    gm_d = din("gm", [1, D], bf16)          # gamma row
    bt_d = din("bt", [1, D], bf16)          # beta row
    e128_d = din("e128", [KB, KB], bf16)    # block-broadcast matrix
    # dense exp(bias): row = (mh, t, partition), col = (h4, j, q)
    F_d = din("F", [2 * NPAIR * KB, 4 * 2 * Q], bf16)
    outT = nc.dram_tensor("outT", [D, Q], f32, kind="ExternalOutput").ap()

    hqT_d = din("hqT", [D, Q], bf16)        # h^T core query slice

    with tile.TileContext(nc) as tc:
        from contextlib import ExitStack

        with ExitStack() as ctx:
            cpool = ctx.enter_context(tc.tile_pool(name="consts", bufs=1))

            def ctile(shape, dt, tag):
                return cpool.tile(shape, dt, tag=tag, name=tag)

            # persistent SBUF residents
            hT = [ctile([KB, N], bf16, f"hT{c}") for c in range(2)]
            hq = [ctile([KB, Q], bf16, f"hq{c}") for c in range(2)]
            xq = [ctile([KB, Q], f32, f"xq{c}") for c in range(2)]
            wq = [ctile([KB, D], bf16, f"wq{c}") for c in range(2)]
            wk = [ctile([KB, D], bf16, f"wk{c}") for c in range(2)]
            wv = [ctile([KB, D], bf16, f"wv{c}") for c in range(2)]
            wo = [ctile([KB, D], bf16, f"wo{c}") for c in range(2)]
            bq = [ctile([KB, 1], f32, f"bq{c}") for c in range(2)]
            bk = [ctile([KB, 1], f32, f"bk{c}") for c in range(2)]
            bo = [ctile([KB, 1], f32, f"bo{c}") for c in range(2)]
            bv_r = ctile([1, D], bf16, "bv_r")
            gm = ctile([1, D], bf16, "gm")
            bt = ctile([1, D], bf16, "bt")
            e128 = ctile([KB, KB], bf16, "e128")
            kT = [ctile([KB, N], bf16, f"kT{c}") for c in range(2)]
            qTb = [ctile([KB, Q], bf16, f"qTb{c}") for c in range(2)]
            # V with ones column: [key, kb, h, 32 dims + 1 one]
            vSB = ctile([KB, NKB, H, DH + 1], bf16, "vSB")
            attnT = [ctile([KB, Q], bf16, f"attnT{c}") for c in range(2)]
            ones_1x128 = ctile([1, KB], bf16, "o1x128")
            ones_128x1 = ctile([KB, 1], bf16, "o128x1")
            ones_1xQ = ctile([1, Q], bf16, "o1xQ")
            epsT = ctile([1, 1], f32, "epsT")
            zer_1xQ = ctile([1, Q], f32, "z1xQ")

            # ---- loads ----
            # DMA priority: Q-proj inputs first, then per-slab hT chunks
            # (both halves per slab so K-proj slab s starts ASAP)
            for c in range(2):
                sl = slice(c * KB, (c + 1) * KB)
                nc.sync.dma_start(out=wq[c][:], in_=Wq_d[sl, :])
                nc.sync.dma_start(out=hq[c][:], in_=hqT_d[sl, :])
                nc.sync.dma_start(out=wk[c][:], in_=Wk_d[sl, :])
                nc.sync.dma_start(out=wv[c][:], in_=Wv_d[sl, :])
                nc.sync.dma_start(out=bq[c][:], in_=bq_d[sl, :])
                nc.sync.dma_start(out=bk[c][:], in_=bk_d[sl, :])
            for s in range(8):
                for c in range(2):
                    row = (c * 8 + s) * KB
                    nc.sync.dma_start(
                        out=hT[c][:, s * Q:(s + 1) * Q],
                        in_=hT_d[row:row + KB, :],
                    )
            nc.sync.dma_start(out=bv_r[:], in_=bv_d[:])
            nc.sync.dma_start(out=e128[:], in_=e128_d[:])
            for c in range(2):
                sl = slice(c * KB, (c + 1) * KB)
                nc.sync.dma_start(out=wo[c][:], in_=Wo_d[sl, :])
                nc.sync.dma_start(out=xq[c][:], in_=xqT_d[sl, :])
                nc.sync.dma_start(out=bo[c][:], in_=bo_d[sl, :])
            nc.sync.dma_start(out=gm[:], in_=gm_d[:])
            nc.sync.dma_start(out=bt[:], in_=bt_d[:])
            nc.vector.memset(ones_1x128[:], 1.0)
            nc.vector.memset(ones_128x1[:], 1.0)
            nc.vector.memset(epsT[:], LN_EPS)
            nc.vector.memset(zer_1xQ[:], 0.0)

            # preload the exp ACT table during projections so the first real
            # exp doesn't stall the attention pipeline for ~2.7us; the output
            # is live (ones_1xQ = exp(0)) so DCE keeps it.
            nc.scalar.activation(ones_1xQ[:], zer_1xQ[:], EXP)

            # ---- projections (all bf16, biases fused) ----
            with tc.tile_pool(name="pps", bufs=3, space="PSUM") as pps:
                IDENT = mybir.ActivationFunctionType.Identity
                # Q^T [2][128, 512] head-major partitions; bias fused into
                # the ACT copy (per-partition bias is native there)
                for mh in range(2):
                    ps = pps.tile([KB, Q], f32, tag="proj", name="proj")
                    for kc in range(2):
                        nc.tensor.matmul(
                            ps[:], wq[kc][:, mh * KB:(mh + 1) * KB], hq[kc][:],
                            start=(kc == 0), stop=(kc == 1),
                        )
                    nc.scalar.activation(
                        qTb[mh][:], ps[:], IDENT, bias=bq[mh][:]
                    )
                # K^T [2][128, 4096]  (interleave mh so attention t=0 for
                # both halves unblocks early)
                for s in range(8):
                    for mh in range(2):
                        ssl = slice(s * Q, (s + 1) * Q)
                        ps = pps.tile([KB, Q], f32, tag="proj", name="proj")
                        for kc in range(2):
                            nc.tensor.matmul(
                                ps[:], wk[kc][:, mh * KB:(mh + 1) * KB],
                                hT[kc][:, ssl],
                                start=(kc == 0), stop=(kc == 1),
                            )
                        nc.scalar.activation(
                            kT[mh][:, ssl], ps[:], IDENT, bias=bk[mh][:]
                        )
                # V node-major [128, kb, h, 33] + bias via rank-1
                nc.vector.memset(vSB[:, :, :, DH], 1.0)
                for kb_i in range(NKB):
                    ksl = slice(kb_i * KB, (kb_i + 1) * KB)
                    psv = pps.tile([KB, D], f32, tag="projv", name="projv")
                    for kc in range(2):
                        nc.tensor.matmul(
                            psv[:], hT[kc][:, ksl], wv[kc][:],
                            start=(kc == 0),
                            stop=(kc == 1 and skip_bias),
                        )
                    if not skip_bias:
                        nc.tensor.matmul(
                            psv[:], ones_1x128[:], bv_r[:],
                            start=False, stop=True,
                        )
                    nc.vector.tensor_copy(vSB[:, kb_i, :, 0:DH], psv[:])

            # ---- attention ----
            with ExitStack() as actx:
                sps = actx.enter_context(
                    tc.tile_pool(name="sps", bufs=3, space="PSUM")
                )
                ops = actx.enter_context(
                    tc.tile_pool(name="ops", bufs=2, space="PSUM")
                )
                spool = actx.enter_context(tc.tile_pool(name="spool", bufs=6))
                fpool = actx.enter_context(tc.tile_pool(name="fpool", bufs=3))
                npool = actx.enter_context(tc.tile_pool(name="npool", bufs=2))

                norm_tail = []
                for mh in range(2):
                    # one oacc tile per head pair: partitions 0:33 head A
                    # (32 dims + z), 64:97 head B.
                    oacc = [
                        ops.tile([KB, Q], f32, tag="oacc", name="oacc")
                        for _ in range(2)
                    ]

                    def issue_pv(g):
                        t, pr, pf = g
                        first = (t == 0)
                        last = (t == NPAIR - 1)
                        for j in range(2):
                            kb_i = 2 * t + j
                            for hp in range(2):
                                h4 = 2 * pr + hp
                                h = 4 * mh + h4
                                nc.tensor.matmul(
                                    oacc[pr][64 * hp:64 * hp + DH + 1, :],
                                    vSB[:, kb_i, h, :],
                                    pf[j][:, hp * Q:(hp + 1) * Q],
                                    start=(first and j == 0),
                                    stop=(last and j == 1),
                                    tile_position=(0, 64 * hp),
                                    skip_group_check=True,
                                )

                    # HAM warm-up: dependency-free burst of matmuls into the
                    # oacc region; the first real PV starts with start=True
                    # so the garbage is overwritten.
                    for w in range(WARM_START):
                        nc.tensor.matmul(
                            oacc[0][0:DH + 1, :], vSB[:, 0, 0, :], qTb[mh][:],
                            start=True, stop=True,
                            tile_position=(0, 0), skip_group_check=True,
                        )
                    # software pipeline, lag 2: PV(g-2) issues BEFORE QK(g)
                    # so the in-order PE always has dependency-free work.
                    from collections import deque
                    pend = deque()
                    for t in range(NPAIR):
                        if WARM_PERIOD and mh == 0 and t == WARM_PERIOD:
                            # periodic dense matmul burst to re-flip HAM
                            bt_ps = sps.tile([KB, 2 * Q], f32, tag="sg",
                                             name="warm")
                            for w in range(WARM_LEN):
                                nc.tensor.matmul(
                                    bt_ps[:, 0:Q],
                                    kT[mh][0:32, 0:KB], qTb[mh][0:32, :],
                                    start=True, stop=True,
                                    tile_position=(0, 0),
                                    skip_group_check=True,
                                )
                        # one 1 MiB F transfer covers (mh, t) x 4 heads
                        fbig = fpool.tile([KB, 8 * Q], bf16, tag="ft",
                                          name="ft")
                        row = (mh * NPAIR + t) * KB
                        nc.sync.dma_start(
                            out=fbig[:], in_=F_d[row:row + KB, :]
                        )
                        for pr in range(2):      # head pairs (2p, 2p+1)
                            if len(pend) >= 2:
                                issue_pv(pend.popleft())
                            # score tiles pair TWO HEADS at the same j so the
                            # two QK matmuls of a tile land on different PE
                            # row groups and run concurrently (row packing).
                            sg = [
                                sps.tile([KB, 2 * Q], f32, tag="sg", name="sg")
                                for _ in range(2)
                            ]
                            # density boosters (optional): dummy weight loads
                            for w in range(WARM_BOOST):
                                nc.tensor.ldweights(
                                    kT[mh][:, 0:KB], tile_position=(0, 0),
                                )
                            for j in range(2):
                                kb_i = 2 * t + j
                                ksl = slice(kb_i * KB, (kb_i + 1) * KB)
                                for hp in range(2):
                                    h4 = 2 * pr + hp
                                    psl = slice(32 * h4, 32 * h4 + 32)
                                    nc.tensor.matmul(
                                        sg[j][:, hp * Q:(hp + 1) * Q],
                                        kT[mh][psl, ksl],
                                        qTb[mh][psl, :],
                                        start=True, stop=True,
                                        tile_position=(32 * h4, 0),
                                    )
                            pf = [None, None]
                            for j in range(2):
                                # exp (ACT) PSUM -> SBUF bf16
                                p0 = spool.tile(
                                    [KB, 2 * Q], bf16, tag="p0", name="p0"
                                )
                                nc.scalar.activation(
                                    p0[:], sg[j][:], EXP, scale=SCALE
                                )
                                pf[j] = spool.tile(
                                    [KB, 2 * Q], bf16, tag="pf", name="pf"
                                )
                                nc.vector.tensor_mul(
                                    pf[j][:], p0[:],
                                    fbig[:, (pr * 2 + j) * 2 * Q:
                                         (pr * 2 + j + 1) * 2 * Q],
                                )
                            pend.append((t, pr, pf))
                    while pend:
                        issue_pv(pend.popleft())

                    # ---- normalize, DVE part: compact numerators + 1/z ----
                    # (the PE-dependent broadcast matmul is deferred for mh0
                    # so it doesn't block mh1's QKs in the in-order PE queue)
                    if mh == 0:
                        # copy to SBUF, releasing oacc PSUM for mh1's PV
                        oaccS = [
                            npool.tile([KB, Q], f32, tag=f"oaccS{pr}",
                                       name=f"oaccS{pr}")
                            for pr in range(2)
                        ]
                        for pr in range(2):
                            nc.vector.tensor_copy(oaccS[pr][:], oacc[pr][:])
                        src = oaccS
                    else:
                        src = oacc  # tail: read PSUM directly
                    onum = npool.tile([KB, Q], f32, tag=f"onum{mh}",
                                      name=f"onum{mh}")
                    zsb = npool.tile([KB, Q], f32, tag="zsb", name="zsb")
                    nc.vector.memset(zsb[:], 1.0)
                    for h4 in range(4):
                        pr, hp = h4 >> 1, h4 & 1
                        nc.vector.tensor_copy(
                            onum[32 * h4:32 * h4 + 32, :],
                            src[pr][64 * hp:64 * hp + 32, :],
                        )
                        nc.vector.tensor_copy(
                            zsb[32 * h4:32 * h4 + 1, :],
                            src[pr][64 * hp + 32:64 * hp + 33, :],
                        )
                    rzb = npool.tile([KB, Q], bf16, tag=f"rzb{mh}",
                                     name=f"rzb{mh}")
                    if mh == 0:
                        # DVE reciprocal (ACT is saturated by the exp stream)
                        rz = npool.tile([KB, Q], f32, tag="rz", name="rz")
                        nc.vector.reciprocal_approx_fast(rz[:], zsb[:])
                        nc.vector.tensor_copy(rzb[:], rz[:])
                    else:
                        # tail: 1/z = exp(-ln z) on the now-idle ACT
                        lnz = npool.tile([KB, Q], f32, tag="lnz", name="lnz")
                        nc.scalar.activation(lnz[:], zsb[:], LN)
                        nc.scalar.activation(rzb[:], lnz[:], EXP, scale=-1.0)
                    norm_tail.append((mh, onum, rzb))

                # ---- normalize, PE part (after all attention matmuls) ----
                for mh, onum, rzb in norm_tail:
                    zbp = sps.tile([KB, Q], f32, tag="sg", name="zbp")
                    nc.tensor.matmul(
                        zbp[:], e128[:], rzb[:], start=True, stop=True
                    )
                    # mixed-space TT: exempt from equal-base-partition rule
                    nc.vector.tensor_mul(attnT[mh][:], onum[:], zbp[:])

            # ---- output projection + residual + LayerNorm ----
            with ExitStack() as ectx:
                rps = ectx.enter_context(
                    tc.tile_pool(name="rps", bufs=1, space="PSUM")
                )
                epool = ectx.enter_context(tc.tile_pool(name="epool", bufs=2))
                out2 = [
                    epool.tile([KB, Q], f32, tag=f"out2_{c}", name=f"out2_{c}")
                    for c in range(2)
                ]
                for c in range(2):
                    op_ps = rps.tile([KB, Q], f32, tag="oproj", name="oproj")
                    for mh in range(2):
                        nc.tensor.matmul(
                            op_ps[:],
                            wo[mh][:, c * KB:(c + 1) * KB],
                            attnT[mh][:],
                            start=(mh == 0), stop=(mh == 1),
                        )
                    # out2 = (psum + bo) + x
                    nc.vector.scalar_tensor_tensor(
                        out2[c][:], op_ps[:], bo[c][:], xq[c][:],
                        op0=ADD, op1=ADD,
                    )
                # stats: mu, s2 via ones matmuls (f32)
                ones_f = epool.tile([KB, 1], f32, tag="onesf", name="onesf")
                nc.vector.memset(ones_f[:], 1.0)
                mu_ps = rps.tile([1, Q], f32, tag="mu", name="mu")
                for c in range(2):
                    nc.tensor.matmul(
                        mu_ps[:], ones_f[:], out2[c][:],
                        start=(c == 0), stop=(c == 1),
                        skip_group_check=True,
                    )
                ones_b = epool.tile([KB, 1], bf16, tag="onesb", name="onesb")
                nc.vector.memset(ones_b[:], 1.0)
                s2_ps = rps.tile([1, Q], f32, tag="s2", name="s2")
                for c in range(2):
                    sq = epool.tile([KB, Q], bf16, tag="sq", name="sq")
                    nc.vector.tensor_mul(sq[:], out2[c][:], out2[c][:])
                    nc.tensor.matmul(
                        s2_ps[:], ones_b[:], sq[:],
                        start=(c == 0), stop=(c == 1),
                        skip_group_check=True,
                    )
                mu = epool.tile([1, Q], f32, tag="mu_s", name="mu_s")
                nc.vector.tensor_scalar_mul(mu[:], mu_ps[:], 1.0 / D)
                m2 = epool.tile([1, Q], f32, tag="m2", name="m2")
                nc.vector.tensor_mul(m2[:], mu[:], mu[:])
                var = epool.tile([1, Q], f32, tag="var", name="var")
                nc.vector.scalar_tensor_tensor(
                    var[:], s2_ps[:], 1.0 / D, m2[:], op0=MULT, op1=SUB,
                )
                # rstd = exp(-0.5*ln(var+eps)): stays in the exp/ln ACT
                # table set, avoiding a ~3us sqrt table switch
                lv = epool.tile([1, Q], f32, tag="lv", name="lv")
                nc.scalar.activation(lv[:], var[:], LN, bias=epsT[:])
                rstd = epool.tile([1, Q], f32, tag="rstd", name="rstd")
                nc.scalar.activation(rstd[:], lv[:], EXP, scale=-0.5)
                # broadcast tiles via outer products:
                # c1 = gamma (x) rstd ; c2 = beta (x) 1 - gamma (x) (mu*rstd)
                rstd_b = epool.tile([1, Q], bf16, tag="rstdb", name="rstdb")
                nc.vector.tensor_copy(rstd_b[:], rstd[:])
                mr = epool.tile([1, Q], f32, tag="mr", name="mr")
                nc.vector.tensor_mul(mr[:], mu[:], rstd[:])
                mrn = epool.tile([1, Q], bf16, tag="mrn", name="mrn")
                nc.vector.tensor_scalar_mul(mrn[:], mr[:], -1.0)
                for c in range(2):
                    csl = slice(c * KB, (c + 1) * KB)
                    c1p = rps.tile([KB, Q], f32, tag="c1", name="c1")
                    nc.tensor.matmul(
                        c1p[:], gm[:, csl], rstd_b[:], start=True, stop=True
                    )
                    # c2 = gamma (x) (-mu*rstd) + beta (x) 1
                    c2p = rps.tile([KB, Q], f32, tag="c2", name="c2")
                    nc.tensor.matmul(
                        c2p[:], gm[:, csl], mrn[:], start=True, stop=False
                    )
                    nc.tensor.matmul(
                        c2p[:], bt[:, csl], ones_1xQ[:],
                        start=False, stop=True,
                    )
                    t1 = epool.tile([KB, Q], f32, tag="t1", name="t1")
                    nc.vector.tensor_mul(t1[:], out2[c][:], c1p[:])
                    y = epool.tile([KB, Q], f32, tag="y", name="y")
                    nc.vector.tensor_add(y[:], t1[:], c2p[:])
                    nc.sync.dma_start(out=outT[csl, :], in_=y[:])

    nc.compile()
    return nc


def _prep_F(q_idx, k_idx, bias_eh):
    """Dense multiplicative bias F = exp(scattered bias), per core.

    Row-block order matches kernel consumption: [mh, t, h4, partition]."""
    key = q_idx.astype(np.int64) * N + k_idx.astype(np.int64)
    uk, inv = np.unique(key, return_inverse=True)
    acc = np.zeros((len(uk), H), np.float32)
    np.add.at(acc, inv, bias_eh)
    uq = (uk // N).astype(np.int32)
    ukey = (uk % N).astype(np.int32)
    vals16 = np.exp(acc).astype(ml_dtypes.bfloat16).view(np.uint16)

    Fs = []
    for i in range(NCORES):
        sel = (uq >> 9) == i
        q = uq[sel] & (Q - 1)
        k = ukey[sel]
        v = vals16[sel]
        t = k >> 8
        j = (k >> 7) & 1
        p = k & (KB - 1)
        # cols ordered (pr, j, hp, q) to match paired score tiles
        F16 = np.full((2, NPAIR, KB, 2, 2, 2, Q), 0x3F80, np.uint16)
        for h in range(H):
            F16[h >> 2, t, p, (h & 3) >> 1, j, h & 1, q] = v[:, h]
        Fs.append(
            np.ascontiguousarray(F16.reshape(2 * NPAIR * KB, 4 * 2 * Q))
            .view(ml_dtypes.bfloat16)
        )
    return Fs


def kernel(**inputs):
    global LAST_RESULTS, _PROG
    x = np.asarray(inputs["x"], np.float32)
    pos = np.asarray(inputs["pos_encoding"], np.float32)
    ei = np.asarray(inputs["edge_index"])
    et = np.asarray(inputs["edge_types"])
    emb = np.asarray(inputs["edge_emb"], np.float32)
    W = {k: np.asarray(inputs[k], np.float32) for k in ("Wq", "Wk", "Wv", "Wo")}
    b = {k: np.asarray(inputs[k], np.float32).reshape(-1)
         for k in ("bq", "bk", "bv", "bo", "gamma", "beta")}

    bias_eh = emb[et]  # [E, H]
    Fs = _prep_F(ei[0], ei[1], bias_eh)

    skip_bias = bool(np.all(b["bv"] == 0.0))
    pkey = (WARM_START, WARM_BOOST, WARM_PERIOD, WARM_LEN, skip_bias)
    if pkey not in _PROG:
        _PROG[pkey] = build_program(skip_bias=skip_bias)
    nc = _PROG[pkey]

    h = (x + pos).astype(np.float32)
    hT = np.ascontiguousarray(h.T.astype(ml_dtypes.bfloat16))
    # chunked layout (half, slab, part, 512)
    hTc = np.ascontiguousarray(
        hT.reshape(2, KB, 8, Q).transpose(0, 2, 1, 3).reshape(2 * 8 * KB, Q)
    )
    xT = np.ascontiguousarray(x.T)
    Wb = {k: np.ascontiguousarray(w.astype(ml_dtypes.bfloat16))
          for k, w in W.items()}
    col = lambda a: np.ascontiguousarray(a.reshape(D, 1))
    row16 = lambda a: np.ascontiguousarray(
        a.reshape(1, D).astype(ml_dtypes.bfloat16)
    )
    e128 = np.zeros((KB, KB), np.float32)
    for h4 in range(4):
        e128[32 * h4, 32 * h4:32 * h4 + 32] = 1.0
    e128 = np.ascontiguousarray(e128.astype(ml_dtypes.bfloat16))

    in_maps = []
    for i in range(NCORES):
        sl = slice(i * Q, (i + 1) * Q)
        in_maps.append({
            "hT": hTc,
            "hqT": np.ascontiguousarray(hT[:, sl]),
            "xqT": np.ascontiguousarray(xT[:, sl]),
            "Wq": Wb["Wq"], "Wk": Wb["Wk"], "Wv": Wb["Wv"], "Wo": Wb["Wo"],
            "bq": col(b["bq"]), "bk": col(b["bk"]), "bo": col(b["bo"]),
            "bv": row16(b["bv"]), "gm": row16(b["gamma"]),
            "bt": row16(b["beta"]), "e128": e128,
            "F": Fs[i],
        })

    trace = os.environ.get("BASS_KERNEL_TRACE", "0") == "1"
    try:
        res = run_bass_kernel_spmd(
            nc, in_maps, list(range(NCORES)), trace=trace
        )
    except Exception:
        if not trace:
            raise
        res = run_bass_kernel_spmd(nc, in_maps, list(range(NCORES)))
    LAST_RESULTS = res

    out = np.empty((N, D), np.float32)
    for i in range(NCORES):
        out[i * Q:(i + 1) * Q, :] = np.asarray(
            res.results[i]["outT"], np.float32
        ).T
    return out


# revision 66
# speedup vs baseline: 1.0227x; 1.0061x over previous
"""Graphormer layer (LocalSubgraphEncoder) Trainium2 Bass kernel, v2.

Sharding: node-parallel over 8 cores. Core i computes the full layer output
for query nodes [512*i, 512*i+512): all 8 heads of attention over all 4096
key nodes, edge-type bias, softmax, output projection, residual, LayerNorm.
No cross-core communication; host concatenates row slices.

v2 design (from perfetto analysis of v1: PE saturated by unpacked K=32
matmuls, GPSIMD dense local_scatter, STT stuck in 1x mode):
 - all matmuls bf16; 2-head row-packing for QK (tile_position row groups)
   and 2-head column-packing for PV / denominator matmuls.
 - scores layout S^T [keys(part), queries(free)]: softmax denominator z
   comes from a packed ones-vector matmul into a shared PSUM bank.
 - edge bias applied multiplicatively AFTER exp: P = exp(S) * F where
   F = exp(scattered bias) is precomputed DENSE on the host and streamed
   from HBM (33.5 MB/core) -> one 2x-mode DVE tensor_tensor per tile;
   GPSIMD does nothing.
 - ACT (ScalarE) does exclusively the exp drain PSUM->SBUF bf16 in
   [128,1024] tiles: the ~128 us floor every design shares.
 - biases fused into DVE copies (per-partition scalar AP) or rank-1 PE
   matmuls; LayerNorm scale/shift via outer-product matmuls.
"""
import os
import sys
import math
import numpy as np

sys.path.insert(0, "/opt/trn_rl_repo")
import ml_dtypes  # noqa: E402
from concourse import bacc, bass, mybir, tile  # noqa: E402
from concourse.bass_utils import run_bass_kernel_spmd  # noqa: E402

N, D, H, E, NT = 4096, 256, 8, 131072, 16
DH = D // H            # 32
NCORES = 8
Q = N // NCORES        # 512 query nodes per core
KB = 128               # key-node block (partition dim)
NKB = N // KB          # 32
NPAIR = NKB // 2       # 16 (two key-blocks per [128,1024] score tile)
LN_EPS = 1e-5
SCALE = 1.0 / math.sqrt(DH)

f32 = mybir.dt.float32
bf16 = mybir.dt.bfloat16
EXP = mybir.ActivationFunctionType.Exp
LN = mybir.ActivationFunctionType.Ln
ADD = mybir.AluOpType.add
MULT = mybir.AluOpType.mult
SUB = mybir.AluOpType.subtract

_PROG = {}
LAST_RESULTS = None

WARM_START = int(os.environ.get("WARM_START", "0"))
WARM_BOOST = int(os.environ.get("WARM_BOOST", "0"))
WARM_PERIOD = int(os.environ.get("WARM_PERIOD", "0"))
WARM_LEN = int(os.environ.get("WARM_LEN", "8"))


_TABLES_PATCHED = False


def _patch_act_tables():
    """Restrict the ACT table chooser to natural_log_exp_and_others (has
    exp, ln and identity) so the whole kernel needs ONE table load instead
    of bouncing between the exp and ln sets. Keys/order preserved so
    act_func_set_id indices stay valid."""
    global _TABLES_PATCHED
    if _TABLES_PATCHED:
        return
    from concourse import hw_specs
    import concourse.bacc as bacc_mod

    orig = hw_specs.get_activation_tables

    def patched(arch):
        t = orig(arch)
        keep = "natural_log_exp_and_others"
        if keep not in t:
            return t
        return {k: (v if k == keep else set()) for k, v in t.items()}

    bacc_mod.get_activation_tables = patched
    _TABLES_PATCHED = True


def build_program(skip_bias=False):
    _patch_act_tables()
    nc = bacc.Bacc(
        "TRN2", target_bir_lowering=False, debug=False, num_devices=NCORES
    )

    def din(name, shape, dt):
        return nc.dram_tensor(name, shape, dt, kind="ExternalInput").ap()

    # (x + pos)^T stored as (half, slab, part, 512) so each chunk is a
    # contiguous 128 KiB transfer and projections start on the first chunk
    hT_d = din("hT", [2 * 8 * KB, Q], bf16)
    xqT_d = din("xqT", [D, Q], f32)         # x^T core slice (residual)
    Wq_d = din("Wq", [D, D], bf16)
    Wk_d = din("Wk", [D, D], bf16)
    Wv_d = din("Wv", [D, D], bf16)
    Wo_d = din("Wo", [D, D], bf16)
    bq_d = din("bq", [D, 1], f32)
    bk_d = din("bk", [D, 1], f32)
    bo_d = din("bo", [D, 1], f32)
    bv_d = din("bv", [1, D], bf16)
    gm_d = din("gm", [1, D], bf16)          # gamma row
    bt_d = din("bt", [1, D], bf16)          # beta row
    e128_d = din("e128", [KB, KB], bf16)    # block-broadcast matrix
    # dense exp(bias): row = (mh, t, partition), col = (h4, j, q)
    F_d = din("F", [2 * NPAIR * KB, 4 * 2 * Q], bf16)
    outT = nc.dram_tensor("outT", [D, Q], f32, kind="ExternalOutput").ap()

    hqT_d = din("hqT", [D, Q], bf16)        # h^T core query slice

    with tile.TileContext(nc) as tc:
        from contextlib import ExitStack

        with ExitStack() as ctx:
            cpool = ctx.enter_context(tc.tile_pool(name="consts", bufs=1))

            def ctile(shape, dt, tag):
                return cpool.tile(shape, dt, tag=tag, name=tag)

            # persistent SBUF residents
            hT = [ctile([KB, N], bf16, f"hT{c}") for c in range(2)]
            hq = [ctile([KB, Q], bf16, f"hq{c}") for c in range(2)]
            xq = [ctile([KB, Q], f32, f"xq{c}") for c in range(2)]
            wq = [ctile([KB, D], bf16, f"wq{c}") for c in range(2)]
            wk = [ctile([KB, D], bf16, f"wk{c}") for c in range(2)]
            wv = [ctile([KB, D], bf16, f"wv{c}") for c in range(2)]
            wo = [ctile([KB, D], bf16, f"wo{c}") for c in range(2)]
            bq = [ctile([KB, 1], f32, f"bq{c}") for c in range(2)]
            bk = [ctile([KB, 1], f32, f"bk{c}") for c in range(2)]
            bo = [ctile([KB, 1], f32, f"bo{c}") for c in range(2)]
            bv_r = ctile([1, D], bf16, "bv_r")
            gm = ctile([1, D], bf16, "gm")
            bt = ctile([1, D], bf16, "bt")
            e128 = ctile([KB, KB], bf16, "e128")
            kT = [ctile([KB, N], bf16, f"kT{c}") for c in range(2)]
            qTb = [ctile([KB, Q], bf16, f"qTb{c}") for c in range(2)]
            # V with ones column: [key, kb, h, 32 dims + 1 one]
            vSB = ctile([KB, NKB, H, DH + 1], bf16, "vSB")
            attnT = [ctile([KB, Q], bf16, f"attnT{c}") for c in range(2)]
            ones_1x128 = ctile([1, KB], bf16, "o1x128")
            ones_128x1 = ctile([KB, 1], bf16, "o128x1")
            ones_1xQ = ctile([1, Q], bf16, "o1xQ")
            epsT = ctile([1, 1], f32, "epsT")
            zer_1xQ = ctile([1, Q], f32, "z1xQ")

            # ---- loads ----
            # DMA priority: Q-proj inputs first, then per-slab hT chunks
            # (both halves per slab so K-proj slab s starts ASAP)
            for c in range(2):
                sl = slice(c * KB, (c + 1) * KB)
                nc.sync.dma_start(out=wq[c][:], in_=Wq_d[sl, :])
                nc.sync.dma_start(out=hq[c][:], in_=hqT_d[sl, :])
                nc.sync.dma_start(out=wk[c][:], in_=Wk_d[sl, :])
                nc.sync.dma_start(out=wv[c][:], in_=Wv_d[sl, :])
                nc.sync.dma_start(out=bq[c][:], in_=bq_d[sl, :])
                nc.sync.dma_start(out=bk[c][:], in_=bk_d[sl, :])
            for s in range(8):
                for c in range(2):
                    row = (c * 8 + s) * KB
                    nc.sync.dma_start(
                        out=hT[c][:, s * Q:(s + 1) * Q],
                        in_=hT_d[row:row + KB, :],
                    )
            nc.sync.dma_start(out=bv_r[:], in_=bv_d[:])
            nc.sync.dma_start(out=e128[:], in_=e128_d[:])
            for c in range(2):
                sl = slice(c * KB, (c + 1) * KB)
                nc.sync.dma_start(out=wo[c][:], in_=Wo_d[sl, :])
                nc.sync.dma_start(out=xq[c][:], in_=xqT_d[sl, :])
                nc.sync.dma_start(out=bo[c][:], in_=bo_d[sl, :])
            nc.sync.dma_start(out=gm[:], in_=gm_d[:])
            nc.sync.dma_start(out=bt[:], in_=bt_d[:])
            nc.vector.memset(ones_1x128[:], 1.0)
            nc.vector.memset(ones_128x1[:], 1.0)
            nc.vector.memset(epsT[:], LN_EPS)
            nc.vector.memset(zer_1xQ[:], 0.0)

            # preload the exp ACT table during projections so the first real
            # exp doesn't stall the attention pipeline for ~2.7us; the output
            # is live (ones_1xQ = exp(0)) so DCE keeps it.
            nc.scalar.activation(ones_1xQ[:], zer_1xQ[:], EXP)

            # ---- projections (all bf16, biases fused) ----
            with tc.tile_pool(name="pps", bufs=3, space="PSUM") as pps:
                IDENT = mybir.ActivationFunctionType.Identity
                # Q^T [2][128, 512] head-major partitions; bias fused into
                # the ACT copy (per-partition bias is native there)
                for mh in range(2):
                    ps = pps.tile([KB, Q], f32, tag="proj", name="proj")
                    for kc in range(2):
                        nc.tensor.matmul(
                            ps[:], wq[kc][:, mh * KB:(mh + 1) * KB], hq[kc][:],
                            start=(kc == 0), stop=(kc == 1),
                        )
                    nc.scalar.activation(
                        qTb[mh][:], ps[:], IDENT, bias=bq[mh][:]
                    )
                # K^T [2][128, 4096]  (interleave mh so attention t=0 for
                # both halves unblocks early)
                for s in range(8):
                    for mh in range(2):
                        ssl = slice(s * Q, (s + 1) * Q)
                        ps = pps.tile([KB, Q], f32, tag="proj", name="proj")
                        for kc in range(2):
                            nc.tensor.matmul(
                                ps[:], wk[kc][:, mh * KB:(mh + 1) * KB],
                                hT[kc][:, ssl],
                                start=(kc == 0), stop=(kc == 1),
                            )
                        nc.scalar.activation(
                            kT[mh][:, ssl], ps[:], IDENT, bias=bk[mh][:]
                        )
                # V node-major [128, kb, h, 33] + bias via rank-1
                nc.vector.memset(vSB[:, :, :, DH], 1.0)
                for kb_i in range(NKB):
                    ksl = slice(kb_i * KB, (kb_i + 1) * KB)
                    psv = pps.tile([KB, D], f32, tag="projv", name="projv")
                    for kc in range(2):
                        nc.tensor.matmul(
                            psv[:], hT[kc][:, ksl], wv[kc][:],
                            start=(kc == 0),
                            stop=(kc == 1 and skip_bias),
                        )
                    if not skip_bias:
                        nc.tensor.matmul(
                            psv[:], ones_1x128[:], bv_r[:],
                            start=False, stop=True,
                        )
                    nc.vector.tensor_copy(vSB[:, kb_i, :, 0:DH], psv[:])

            # ---- attention ----
            with ExitStack() as actx:
                sps = actx.enter_context(
                    tc.tile_pool(name="sps", bufs=3, space="PSUM")
                )
                ops = actx.enter_context(
                    tc.tile_pool(name="ops", bufs=2, space="PSUM")
                )
                spool = actx.enter_context(tc.tile_pool(name="spool", bufs=6))
                fpool = actx.enter_context(tc.tile_pool(name="fpool", bufs=3))
                npool = actx.enter_context(tc.tile_pool(name="npool", bufs=2))

                norm_tail = []
                for mh in range(2):
                    # one oacc tile per head pair: partitions 0:33 head A
                    # (32 dims + z), 64:97 head B.
                    oacc = [
                        ops.tile([KB, Q], f32, tag="oacc", name="oacc")
                        for _ in range(2)
                    ]

                    def issue_pv(g):
                        t, pr, pf = g
                        first = (t == 0)
                        last = (t == NPAIR - 1)
                        for j in range(2):
                            kb_i = 2 * t + j
                            for hp in range(2):
                                h4 = 2 * pr + hp
                                h = 4 * mh + h4
                                nc.tensor.matmul(
                                    oacc[pr][64 * hp:64 * hp + DH + 1, :],
                                    vSB[:, kb_i, h, :],
                                    pf[j][:, hp * Q:(hp + 1) * Q],
                                    start=(first and j == 0),
                                    stop=(last and j == 1),
                                    tile_position=(0, 64 * hp),
                                    skip_group_check=True,
                                )

                    # HAM warm-up: dependency-free burst of matmuls into the
                    # oacc region; the first real PV starts with start=True
                    # so the garbage is overwritten.
                    for w in range(WARM_START):
                        nc.tensor.matmul(
                            oacc[0][0:DH + 1, :], vSB[:, 0, 0, :], qTb[mh][:],
                            start=True, stop=True,
                            tile_position=(0, 0), skip_group_check=True,
                        )
                    # software pipeline, lag 2: PV(g-2) issues BEFORE QK(g)
                    # so the in-order PE always has dependency-free work.
                    from collections import deque
                    pend = deque()
                    for t in range(NPAIR):
                        if WARM_PERIOD and mh == 0 and t == WARM_PERIOD:
                            # periodic dense matmul burst to re-flip HAM
                            bt_ps = sps.tile([KB, 2 * Q], f32, tag="sg",
                                             name="warm")
                            for w in range(WARM_LEN):
                                nc.tensor.matmul(
                                    bt_ps[:, 0:Q],
                                    kT[mh][0:32, 0:KB], qTb[mh][0:32, :],
                                    start=True, stop=True,
                                    tile_position=(0, 0),
                                    skip_group_check=True,
                                )
                        # one 1 MiB F transfer covers (mh, t) x 4 heads
                        fbig = fpool.tile([KB, 8 * Q], bf16, tag="ft",
                                          name="ft")
                        row = (mh * NPAIR + t) * KB
                        nc.sync.dma_start(
                            out=fbig[:], in_=F_d[row:row + KB, :]
                        )
                        for pr in range(2):      # head pairs (2p, 2p+1)
                            if len(pend) >= 2:
                                issue_pv(pend.popleft())
                            # score tiles pair TWO HEADS at the same j so the
                            # two QK matmuls of a tile land on different PE
                            # row groups and run concurrently (row packing).
                            sg = [
                                sps.tile([KB, 2 * Q], f32, tag="sg", name="sg")
                                for _ in range(2)
                            ]
                            # density boosters (optional): dummy weight loads
                            for w in range(WARM_BOOST):
                                nc.tensor.ldweights(
                                    kT[mh][:, 0:KB], tile_position=(0, 0),
                                )
                            for j in range(2):
                                kb_i = 2 * t + j
                                ksl = slice(kb_i * KB, (kb_i + 1) * KB)
                                for hp in range(2):
                                    h4 = 2 * pr + hp
                                    psl = slice(32 * h4, 32 * h4 + 32)
                                    nc.tensor.matmul(
                                        sg[j][:, hp * Q:(hp + 1) * Q],
                                        kT[mh][psl, ksl],
                                        qTb[mh][psl, :],
                                        start=True, stop=True,
                                        tile_position=(32 * h4, 0),
                                    )
                            pf = [None, None]
                            for j in range(2):
                                # exp (ACT) PSUM -> SBUF bf16
                                p0 = spool.tile(
                                    [KB, 2 * Q], bf16, tag="p0", name="p0"
                                )
                                nc.scalar.activation(
                                    p0[:], sg[j][:], EXP, scale=SCALE
                                )
                                pf[j] = spool.tile(
                                    [KB, 2 * Q], bf16, tag="pf", name="pf"
                                )
                                nc.vector.tensor_mul(
                                    pf[j][:], p0[:],
                                    fbig[:, (pr * 2 + j) * 2 * Q:
                                         (pr * 2 + j + 1) * 2 * Q],
                                )
                            pend.append((t, pr, pf))
                    while pend:
                        issue_pv(pend.popleft())

                    # ---- normalize, DVE part: compact numerators + 1/z ----
                    # (the PE-dependent broadcast matmul is deferred for mh0
                    # so it doesn't block mh1's QKs in the in-order PE queue)
                    if mh == 0:
                        # copy to SBUF, releasing oacc PSUM for mh1's PV
                        oaccS = [
                            npool.tile([KB, Q], f32, tag=f"oaccS{pr}",
                                       name=f"oaccS{pr}")
                            for pr in range(2)
                        ]
                        for pr in range(2):
                            nc.vector.tensor_copy(oaccS[pr][:], oacc[pr][:])
                        src = oaccS
                    else:
                        src = oacc  # tail: read PSUM directly
                    onum = npool.tile([KB, Q], f32, tag=f"onum{mh}",
                                      name=f"onum{mh}")
                    zsb = npool.tile([KB, Q], f32, tag="zsb", name="zsb")
                    nc.vector.memset(zsb[:], 1.0)
                    IDENT = mybir.ActivationFunctionType.Identity
                    for h4 in range(4):
                        pr, hp = h4 >> 1, h4 & 1
                        if mh == 1:
                            # tail: ACT is idle; run compaction there so it
                            # overlaps the DVE z-gather
                            nc.scalar.activation(
                                onum[32 * h4:32 * h4 + 32, :],
                                src[pr][64 * hp:64 * hp + 32, :], IDENT,
                            )
                        else:
                            nc.vector.tensor_copy(
                                onum[32 * h4:32 * h4 + 32, :],
                                src[pr][64 * hp:64 * hp + 32, :],
                            )
                        nc.vector.tensor_copy(
                            zsb[32 * h4:32 * h4 + 1, :],
                            src[pr][64 * hp + 32:64 * hp + 33, :],
                        )
                    rzb = npool.tile([KB, Q], bf16, tag=f"rzb{mh}",
                                     name=f"rzb{mh}")
                    if mh == 0:
                        # DVE reciprocal (ACT is saturated by the exp stream)
                        rz = npool.tile([KB, Q], f32, tag="rz", name="rz")
                        nc.vector.reciprocal_approx_fast(rz[:], zsb[:])
                        nc.vector.tensor_copy(rzb[:], rz[:])
                    else:
                        # tail: 1/z = exp(-ln z) on the now-idle ACT
                        lnz = npool.tile([KB, Q], f32, tag="lnz", name="lnz")
                        nc.scalar.activation(lnz[:], zsb[:], LN)
                        nc.scalar.activation(rzb[:], lnz[:], EXP, scale=-1.0)
                    norm_tail.append((mh, onum, rzb))

                # ---- normalize, PE part (after all attention matmuls) ----
                for mh, onum, rzb in norm_tail:
                    zbp = sps.tile([KB, Q], f32, tag="sg", name="zbp")
                    nc.tensor.matmul(
                        zbp[:], e128[:], rzb[:], start=True, stop=True
                    )
                    # mixed-space TT: exempt from equal-base-partition rule
                    nc.vector.tensor_mul(attnT[mh][:], onum[:], zbp[:])

            # ---- output projection + residual + LayerNorm ----
            with ExitStack() as ectx:
                rps = ectx.enter_context(
                    tc.tile_pool(name="rps", bufs=1, space="PSUM")
                )
                epool = ectx.enter_context(tc.tile_pool(name="epool", bufs=2))
                out2 = [
                    epool.tile([KB, Q], f32, tag=f"out2_{c}", name=f"out2_{c}")
                    for c in range(2)
                ]
                for c in range(2):
                    op_ps = rps.tile([KB, Q], f32, tag="oproj", name="oproj")
                    for mh in range(2):
                        nc.tensor.matmul(
                            op_ps[:],
                            wo[mh][:, c * KB:(c + 1) * KB],
                            attnT[mh][:],
                            start=(mh == 0), stop=(mh == 1),
                        )
                    # out2 = (psum + bo) + x
                    nc.vector.scalar_tensor_tensor(
                        out2[c][:], op_ps[:], bo[c][:], xq[c][:],
                        op0=ADD, op1=ADD,
                    )
                # stats: mu, s2 via ones matmuls (f32)
                ones_f = epool.tile([KB, 1], f32, tag="onesf", name="onesf")
                nc.vector.memset(ones_f[:], 1.0)
                mu_ps = rps.tile([1, Q], f32, tag="mu", name="mu")
                for c in range(2):
                    nc.tensor.matmul(
                        mu_ps[:], ones_f[:], out2[c][:],
                        start=(c == 0), stop=(c == 1),
                        skip_group_check=True,
                    )
                ones_b = epool.tile([KB, 1], bf16, tag="onesb", name="onesb")
                nc.vector.memset(ones_b[:], 1.0)
                s2_ps = rps.tile([1, Q], f32, tag="s2", name="s2")
                for c in range(2):
                    sq = epool.tile([KB, Q], bf16, tag="sq", name="sq")
                    nc.vector.tensor_mul(sq[:], out2[c][:], out2[c][:])
                    nc.tensor.matmul(
                        s2_ps[:], ones_b[:], sq[:],
                        start=(c == 0), stop=(c == 1),
                        skip_group_check=True,
                    )
                mu = epool.tile([1, Q], f32, tag="mu_s", name="mu_s")
                nc.vector.tensor_scalar_mul(mu[:], mu_ps[:], 1.0 / D)
                m2 = epool.tile([1, Q], f32, tag="m2", name="m2")
                nc.vector.tensor_mul(m2[:], mu[:], mu[:])
                var = epool.tile([1, Q], f32, tag="var", name="var")
                nc.vector.scalar_tensor_tensor(
                    var[:], s2_ps[:], 1.0 / D, m2[:], op0=MULT, op1=SUB,
                )
                # rstd = exp(-0.5*ln(var+eps)): stays in the exp/ln ACT
                # table set, avoiding a ~3us sqrt table switch
                lv = epool.tile([1, Q], f32, tag="lv", name="lv")
                nc.scalar.activation(lv[:], var[:], LN, bias=epsT[:])
                rstd = epool.tile([1, Q], f32, tag="rstd", name="rstd")
                nc.scalar.activation(rstd[:], lv[:], EXP, scale=-0.5)
                # broadcast tiles via outer products:
                # c1 = gamma (x) rstd ; c2 = beta (x) 1 - gamma (x) (mu*rstd)
                rstd_b = epool.tile([1, Q], bf16, tag="rstdb", name="rstdb")
                nc.vector.tensor_copy(rstd_b[:], rstd[:])
                mr = epool.tile([1, Q], f32, tag="mr", name="mr")
                nc.vector.tensor_mul(mr[:], mu[:], rstd[:])
                mrn = epool.tile([1, Q], bf16, tag="mrn", name="mrn")
                nc.vector.tensor_scalar_mul(mrn[:], mr[:], -1.0)
                for c in range(2):
                    csl = slice(c * KB, (c + 1) * KB)
                    c1p = rps.tile([KB, Q], f32, tag="c1", name="c1")
                    nc.tensor.matmul(
                        c1p[:], gm[:, csl], rstd_b[:], start=True, stop=True
                    )
                    # c2 = gamma (x) (-mu*rstd) + beta (x) 1
                    c2p = rps.tile([KB, Q], f32, tag="c2", name="c2")
                    nc.tensor.matmul(
                        c2p[:], gm[:, csl], mrn[:], start=True, stop=False
                    )
                    nc.tensor.matmul(
                        c2p[:], bt[:, csl], ones_1xQ[:],
                        start=False, stop=True,
                    )
                    t1 = epool.tile([KB, Q], f32, tag="t1", name="t1")
                    nc.vector.tensor_mul(t1[:], out2[c][:], c1p[:])
                    y = epool.tile([KB, Q], f32, tag="y", name="y")
                    nc.vector.tensor_add(y[:], t1[:], c2p[:])
                    nc.sync.dma_start(out=outT[csl, :], in_=y[:])

    nc.compile()
    return nc


def _prep_F(q_idx, k_idx, bias_eh):
    """Dense multiplicative bias F = exp(scattered bias), per core.

    Row-block order matches kernel consumption: [mh, t, h4, partition]."""
    key = q_idx.astype(np.int64) * N + k_idx.astype(np.int64)
    uk, inv = np.unique(key, return_inverse=True)
    acc = np.zeros((len(uk), H), np.float32)
    np.add.at(acc, inv, bias_eh)
    uq = (uk // N).astype(np.int32)
    ukey = (uk % N).astype(np.int32)
    vals16 = np.exp(acc).astype(ml_dtypes.bfloat16).view(np.uint16)

    Fs = []
    for i in range(NCORES):
        sel = (uq >> 9) == i
        q = uq[sel] & (Q - 1)
        k = ukey[sel]
        v = vals16[sel]
        t = k >> 8
        j = (k >> 7) & 1
        p = k & (KB - 1)
        # cols ordered (pr, j, hp, q) to match paired score tiles
        F16 = np.full((2, NPAIR, KB, 2, 2, 2, Q), 0x3F80, np.uint16)
        for h in range(H):
            F16[h >> 2, t, p, (h & 3) >> 1, j, h & 1, q] = v[:, h]
        Fs.append(
            np.ascontiguousarray(F16.reshape(2 * NPAIR * KB, 4 * 2 * Q))
            .view(ml_dtypes.bfloat16)
        )
    return Fs


def kernel(**inputs):
    global LAST_RESULTS, _PROG
    x = np.asarray(inputs["x"], np.float32)
    pos = np.asarray(inputs["pos_encoding"], np.float32)
    ei = np.asarray(inputs["edge_index"])
    et = np.asarray(inputs["edge_types"])
    emb = np.asarray(inputs["edge_emb"], np.float32)
    W = {k: np.asarray(inputs[k], np.float32) for k in ("Wq", "Wk", "Wv", "Wo")}
    b = {k: np.asarray(inputs[k], np.float32).reshape(-1)
         for k in ("bq", "bk", "bv", "bo", "gamma", "beta")}

    bias_eh = emb[et]  # [E, H]
    Fs = _prep_F(ei[0], ei[1], bias_eh)

    skip_bias = bool(np.all(b["bv"] == 0.0))
    pkey = (WARM_START, WARM_BOOST, WARM_PERIOD, WARM_LEN, skip_bias)
    if pkey not in _PROG:
        _PROG[pkey] = build_program(skip_bias=skip_bias)
    nc = _PROG[pkey]

    h = (x + pos).astype(np.float32)
    hT = np.ascontiguousarray(h.T.astype(ml_dtypes.bfloat16))
    # chunked layout (half, slab, part, 512)
    hTc = np.ascontiguousarray(
        hT.reshape(2, KB, 8, Q).transpose(0, 2, 1, 3).reshape(2 * 8 * KB, Q)
    )
    xT = np.ascontiguousarray(x.T)
    Wb = {k: np.ascontiguousarray(w.astype(ml_dtypes.bfloat16))
          for k, w in W.items()}
    col = lambda a: np.ascontiguousarray(a.reshape(D, 1))
    row16 = lambda a: np.ascontiguousarray(
        a.reshape(1, D).astype(ml_dtypes.bfloat16)
    )
    e128 = np.zeros((KB, KB), np.float32)
    for h4 in range(4):
        e128[32 * h4, 32 * h4:32 * h4 + 32] = 1.0
    e128 = np.ascontiguousarray(e128.astype(ml_dtypes.bfloat16))

    in_maps = []
    for i in range(NCORES):
        sl = slice(i * Q, (i + 1) * Q)
        in_maps.append({
            "hT": hTc,
            "hqT": np.ascontiguousarray(hT[:, sl]),
            "xqT": np.ascontiguousarray(xT[:, sl]),
            "Wq": Wb["Wq"], "Wk": Wb["Wk"], "Wv": Wb["Wv"], "Wo": Wb["Wo"],
            "bq": col(b["bq"]), "bk": col(b["bk"]), "bo": col(b["bo"]),
            "bv": row16(b["bv"]), "gm": row16(b["gamma"]),
            "bt": row16(b["beta"]), "e128": e128,
            "F": Fs[i],
        })

    trace = os.environ.get("BASS_KERNEL_TRACE", "0") == "1"
    try:
        res = run_bass_kernel_spmd(
            nc, in_maps, list(range(NCORES)), trace=trace
        )
    except Exception:
        if not trace:
            raise
        res = run_bass_kernel_spmd(nc, in_maps, list(range(NCORES)))
    LAST_RESULTS = res

    out = np.empty((N, D), np.float32)
    for i in range(NCORES):
        out[i * Q:(i + 1) * Q, :] = np.asarray(
            res.results[i]["outT"], np.float32
        ).T
    return out


# revision 67
# speedup vs baseline: 1.0230x; 1.0004x over previous
"""Graphormer layer (LocalSubgraphEncoder) Trainium2 Bass kernel, v2.

Sharding: node-parallel over 8 cores. Core i computes the full layer output
for query nodes [512*i, 512*i+512): all 8 heads of attention over all 4096
key nodes, edge-type bias, softmax, output projection, residual, LayerNorm.
No cross-core communication; host concatenates row slices.

v2 design (from perfetto analysis of v1: PE saturated by unpacked K=32
matmuls, GPSIMD dense local_scatter, STT stuck in 1x mode):
 - all matmuls bf16; 2-head row-packing for QK (tile_position row groups)
   and 2-head column-packing for PV / denominator matmuls.
 - scores layout S^T [keys(part), queries(free)]: softmax denominator z
   comes from a packed ones-vector matmul into a shared PSUM bank.
 - edge bias applied multiplicatively AFTER exp: P = exp(S) * F where
   F = exp(scattered bias) is precomputed DENSE on the host and streamed
   from HBM (33.5 MB/core) -> one 2x-mode DVE tensor_tensor per tile;
   GPSIMD does nothing.
 - ACT (ScalarE) does exclusively the exp drain PSUM->SBUF bf16 in
   [128,1024] tiles: the ~128 us floor every design shares.
 - biases fused into DVE copies (per-partition scalar AP) or rank-1 PE
   matmuls; LayerNorm scale/shift via outer-product matmuls.
"""
import os
import sys
import math
import numpy as np

sys.path.insert(0, "/opt/trn_rl_repo")
import ml_dtypes  # noqa: E402
from concourse import bacc, bass, mybir, tile  # noqa: E402
from concourse.bass_utils import run_bass_kernel_spmd  # noqa: E402

N, D, H, E, NT = 4096, 256, 8, 131072, 16
DH = D // H            # 32
NCORES = 8
Q = N // NCORES        # 512 query nodes per core
KB = 128               # key-node block (partition dim)
NKB = N // KB          # 32
NPAIR = NKB // 2       # 16 (two key-blocks per [128,1024] score tile)
LN_EPS = 1e-5
SCALE = 1.0 / math.sqrt(DH)

f32 = mybir.dt.float32
bf16 = mybir.dt.bfloat16
EXP = mybir.ActivationFunctionType.Exp
LN = mybir.ActivationFunctionType.Ln
ADD = mybir.AluOpType.add
MULT = mybir.AluOpType.mult
SUB = mybir.AluOpType.subtract

_PROG = {}
LAST_RESULTS = None

WARM_START = int(os.environ.get("WARM_START", "0"))
WARM_BOOST = int(os.environ.get("WARM_BOOST", "0"))
WARM_PERIOD = int(os.environ.get("WARM_PERIOD", "0"))
WARM_LEN = int(os.environ.get("WARM_LEN", "8"))


_TABLES_PATCHED = False


def _patch_act_tables():
    """Restrict the ACT table chooser to natural_log_exp_and_others (has
    exp, ln and identity) so the whole kernel needs ONE table load instead
    of bouncing between the exp and ln sets. Keys/order preserved so
    act_func_set_id indices stay valid."""
    global _TABLES_PATCHED
    if _TABLES_PATCHED:
        return
    from concourse import hw_specs
    import concourse.bacc as bacc_mod

    orig = hw_specs.get_activation_tables

    def patched(arch):
        t = orig(arch)
        keep = "natural_log_exp_and_others"
        if keep not in t:
            return t
        return {k: (v if k == keep else set()) for k, v in t.items()}

    bacc_mod.get_activation_tables = patched
    _TABLES_PATCHED = True


def build_program(skip_bias=False):
    _patch_act_tables()
    nc = bacc.Bacc(
        "TRN2", target_bir_lowering=False, debug=False, num_devices=NCORES
    )

    def din(name, shape, dt):
        return nc.dram_tensor(name, shape, dt, kind="ExternalInput").ap()

    # (x + pos)^T stored as (half, slab, part, 512) so each chunk is a
    # contiguous 128 KiB transfer and projections start on the first chunk
    hT_d = din("hT", [2 * 8 * KB, Q], bf16)
    xqT_d = din("xqT", [D, Q], f32)         # x^T core slice (residual)
    Wq_d = din("Wq", [D, D], bf16)
    Wk_d = din("Wk", [D, D], bf16)
    Wv_d = din("Wv", [D, D], bf16)
    Wo_d = din("Wo", [D, D], bf16)
    bq_d = din("bq", [D, 1], f32)
    bk_d = din("bk", [D, 1], f32)
    bo_d = din("bo", [D, 1], f32)
    bv_d = din("bv", [1, D], bf16)
    gm_d = din("gm", [1, D], bf16)          # gamma row
    bt_d = din("bt", [1, D], bf16)          # beta row
    e128_d = din("e128", [KB, KB], bf16)    # block-broadcast matrix
    # dense exp(bias): row = (mh, t, partition), col = (h4, j, q)
    F_d = din("F", [2 * NPAIR * KB, 4 * 2 * Q], bf16)
    outT = nc.dram_tensor("outT", [D, Q], f32, kind="ExternalOutput").ap()

    hqT_d = din("hqT", [D, Q], bf16)        # h^T core query slice

    with tile.TileContext(nc) as tc:
        from contextlib import ExitStack

        with ExitStack() as ctx:
            cpool = ctx.enter_context(tc.tile_pool(name="consts", bufs=1))

            def ctile(shape, dt, tag):
                return cpool.tile(shape, dt, tag=tag, name=tag)

            # persistent SBUF residents
            hT = [ctile([KB, N], bf16, f"hT{c}") for c in range(2)]
            hq = [ctile([KB, Q], bf16, f"hq{c}") for c in range(2)]
            xq = [ctile([KB, Q], f32, f"xq{c}") for c in range(2)]
            wq = [ctile([KB, D], bf16, f"wq{c}") for c in range(2)]
            wk = [ctile([KB, D], bf16, f"wk{c}") for c in range(2)]
            wv = [ctile([KB, D], bf16, f"wv{c}") for c in range(2)]
            wo = [ctile([KB, D], bf16, f"wo{c}") for c in range(2)]
            bq = [ctile([KB, 1], f32, f"bq{c}") for c in range(2)]
            bk = [ctile([KB, 1], f32, f"bk{c}") for c in range(2)]
            bo = [ctile([KB, 1], f32, f"bo{c}") for c in range(2)]
            bv_r = ctile([1, D], bf16, "bv_r")
            gm = ctile([1, D], bf16, "gm")
            bt = ctile([1, D], bf16, "bt")
            e128 = ctile([KB, KB], bf16, "e128")
            kT = [ctile([KB, N], bf16, f"kT{c}") for c in range(2)]
            qTb = [ctile([KB, Q], bf16, f"qTb{c}") for c in range(2)]
            # V with ones column: [key, kb, h, 32 dims + 1 one]
            vSB = ctile([KB, NKB, H, DH + 1], bf16, "vSB")
            attnT = [ctile([KB, Q], bf16, f"attnT{c}") for c in range(2)]
            ones_1x128 = ctile([1, KB], bf16, "o1x128")
            ones_128x1 = ctile([KB, 1], bf16, "o128x1")
            ones_1xQ = ctile([1, Q], bf16, "o1xQ")
            epsT = ctile([1, 1], f32, "epsT")
            zer_1xQ = ctile([1, Q], f32, "z1xQ")

            # ---- loads ----
            # DMA priority: Q-proj inputs first, then per-slab hT chunks
            # (both halves per slab so K-proj slab s starts ASAP)
            for c in range(2):
                sl = slice(c * KB, (c + 1) * KB)
                nc.sync.dma_start(out=wq[c][:], in_=Wq_d[sl, :])
                nc.sync.dma_start(out=hq[c][:], in_=hqT_d[sl, :])
                nc.sync.dma_start(out=wk[c][:], in_=Wk_d[sl, :])
                nc.sync.dma_start(out=wv[c][:], in_=Wv_d[sl, :])
                nc.sync.dma_start(out=bq[c][:], in_=bq_d[sl, :])
                nc.sync.dma_start(out=bk[c][:], in_=bk_d[sl, :])
            for s in range(8):
                for c in range(2):
                    row = (c * 8 + s) * KB
                    nc.sync.dma_start(
                        out=hT[c][:, s * Q:(s + 1) * Q],
                        in_=hT_d[row:row + KB, :],
                    )
            nc.sync.dma_start(out=bv_r[:], in_=bv_d[:])
            nc.sync.dma_start(out=e128[:], in_=e128_d[:])
            for c in range(2):
                sl = slice(c * KB, (c + 1) * KB)
                nc.sync.dma_start(out=wo[c][:], in_=Wo_d[sl, :])
                nc.sync.dma_start(out=xq[c][:], in_=xqT_d[sl, :])
                nc.sync.dma_start(out=bo[c][:], in_=bo_d[sl, :])
            nc.sync.dma_start(out=gm[:], in_=gm_d[:])
            nc.sync.dma_start(out=bt[:], in_=bt_d[:])
            nc.vector.memset(ones_1x128[:], 1.0)
            nc.vector.memset(ones_128x1[:], 1.0)
            nc.vector.memset(epsT[:], LN_EPS)
            nc.vector.memset(zer_1xQ[:], 0.0)

            # preload the exp ACT table during projections so the first real
            # exp doesn't stall the attention pipeline for ~2.7us; the output
            # is live (ones_1xQ = exp(0)) so DCE keeps it.
            nc.scalar.activation(ones_1xQ[:], zer_1xQ[:], EXP)

            # ---- projections (all bf16, biases fused) ----
            with tc.tile_pool(name="pps", bufs=3, space="PSUM") as pps:
                IDENT = mybir.ActivationFunctionType.Identity
                # Q^T [2][128, 512] head-major partitions; bias fused into
                # the ACT copy (per-partition bias is native there)
                for mh in range(2):
                    ps = pps.tile([KB, Q], f32, tag="proj", name="proj")
                    for kc in range(2):
                        nc.tensor.matmul(
                            ps[:], wq[kc][:, mh * KB:(mh + 1) * KB], hq[kc][:],
                            start=(kc == 0), stop=(kc == 1),
                        )
                    nc.scalar.activation(
                        qTb[mh][:], ps[:], IDENT, bias=bq[mh][:]
                    )
                # K^T [2][128, 4096]  (interleave mh so attention t=0 for
                # both halves unblocks early)
                for s in range(8):
                    for mh in range(2):
                        ssl = slice(s * Q, (s + 1) * Q)
                        ps = pps.tile([KB, Q], f32, tag="proj", name="proj")
                        for kc in range(2):
                            nc.tensor.matmul(
                                ps[:], wk[kc][:, mh * KB:(mh + 1) * KB],
                                hT[kc][:, ssl],
                                start=(kc == 0), stop=(kc == 1),
                            )
                        nc.scalar.activation(
                            kT[mh][:, ssl], ps[:], IDENT, bias=bk[mh][:]
                        )
                # V node-major [128, kb, h, 33] + bias via rank-1
                nc.vector.memset(vSB[:, :, :, DH], 1.0)
                for kb_i in range(NKB):
                    ksl = slice(kb_i * KB, (kb_i + 1) * KB)
                    psv = pps.tile([KB, D], f32, tag="projv", name="projv")
                    for kc in range(2):
                        nc.tensor.matmul(
                            psv[:], hT[kc][:, ksl], wv[kc][:],
                            start=(kc == 0),
                            stop=(kc == 1 and skip_bias),
                        )
                    if not skip_bias:
                        nc.tensor.matmul(
                            psv[:], ones_1x128[:], bv_r[:],
                            start=False, stop=True,
                        )
                    nc.vector.tensor_copy(vSB[:, kb_i, :, 0:DH], psv[:])

            # ---- attention ----
            with ExitStack() as actx:
                sps = actx.enter_context(
                    tc.tile_pool(name="sps", bufs=3, space="PSUM")
                )
                ops = actx.enter_context(
                    tc.tile_pool(name="ops", bufs=2, space="PSUM")
                )
                spool = actx.enter_context(tc.tile_pool(name="spool", bufs=8))
                fpool = actx.enter_context(tc.tile_pool(name="fpool", bufs=3))
                npool = actx.enter_context(tc.tile_pool(name="npool", bufs=2))

                norm_tail = []
                for mh in range(2):
                    # one oacc tile per head pair: partitions 0:33 head A
                    # (32 dims + z), 64:97 head B.
                    oacc = [
                        ops.tile([KB, Q], f32, tag="oacc", name="oacc")
                        for _ in range(2)
                    ]

                    def issue_pv(g):
                        t, pr, pf = g
                        first = (t == 0)
                        last = (t == NPAIR - 1)
                        for j in range(2):
                            kb_i = 2 * t + j
                            for hp in range(2):
                                h4 = 2 * pr + hp
                                h = 4 * mh + h4
                                nc.tensor.matmul(
                                    oacc[pr][64 * hp:64 * hp + DH + 1, :],
                                    vSB[:, kb_i, h, :],
                                    pf[j][:, hp * Q:(hp + 1) * Q],
                                    start=(first and j == 0),
                                    stop=(last and j == 1),
                                    tile_position=(0, 64 * hp),
                                    skip_group_check=True,
                                )

                    # HAM warm-up: dependency-free burst of matmuls into the
                    # oacc region; the first real PV starts with start=True
                    # so the garbage is overwritten.
                    for w in range(WARM_START):
                        nc.tensor.matmul(
                            oacc[0][0:DH + 1, :], vSB[:, 0, 0, :], qTb[mh][:],
                            start=True, stop=True,
                            tile_position=(0, 0), skip_group_check=True,
                        )
                    # software pipeline, lag 2: PV(g-2) issues BEFORE QK(g)
                    # so the in-order PE always has dependency-free work.
                    from collections import deque
                    pend = deque()
                    for t in range(NPAIR):
                        if WARM_PERIOD and mh == 0 and t == WARM_PERIOD:
                            # periodic dense matmul burst to re-flip HAM
                            bt_ps = sps.tile([KB, 2 * Q], f32, tag="sg",
                                             name="warm")
                            for w in range(WARM_LEN):
                                nc.tensor.matmul(
                                    bt_ps[:, 0:Q],
                                    kT[mh][0:32, 0:KB], qTb[mh][0:32, :],
                                    start=True, stop=True,
                                    tile_position=(0, 0),
                                    skip_group_check=True,
                                )
                        # one 1 MiB F transfer covers (mh, t) x 4 heads
                        fbig = fpool.tile([KB, 8 * Q], bf16, tag="ft",
                                          name="ft")
                        row = (mh * NPAIR + t) * KB
                        nc.sync.dma_start(
                            out=fbig[:], in_=F_d[row:row + KB, :]
                        )
                        for pr in range(2):      # head pairs (2p, 2p+1)
                            if len(pend) >= 2:
                                issue_pv(pend.popleft())
                            # score tiles pair TWO HEADS at the same j so the
                            # two QK matmuls of a tile land on different PE
                            # row groups and run concurrently (row packing).
                            sg = [
                                sps.tile([KB, 2 * Q], f32, tag="sg", name="sg")
                                for _ in range(2)
                            ]
                            # density boosters (optional): dummy weight loads
                            for w in range(WARM_BOOST):
                                nc.tensor.ldweights(
                                    kT[mh][:, 0:KB], tile_position=(0, 0),
                                )
                            for j in range(2):
                                kb_i = 2 * t + j
                                ksl = slice(kb_i * KB, (kb_i + 1) * KB)
                                for hp in range(2):
                                    h4 = 2 * pr + hp
                                    psl = slice(32 * h4, 32 * h4 + 32)
                                    nc.tensor.matmul(
                                        sg[j][:, hp * Q:(hp + 1) * Q],
                                        kT[mh][psl, ksl],
                                        qTb[mh][psl, :],
                                        start=True, stop=True,
                                        tile_position=(32 * h4, 0),
                                    )
                            pf = [None, None]
                            for j in range(2):
                                # exp (ACT) PSUM -> SBUF bf16
                                p0 = spool.tile(
                                    [KB, 2 * Q], bf16, tag="p0", name="p0"
                                )
                                nc.scalar.activation(
                                    p0[:], sg[j][:], EXP, scale=SCALE
                                )
                                pf[j] = spool.tile(
                                    [KB, 2 * Q], bf16, tag="pf", name="pf"
                                )
                                nc.vector.tensor_mul(
                                    pf[j][:], p0[:],
                                    fbig[:, (pr * 2 + j) * 2 * Q:
                                         (pr * 2 + j + 1) * 2 * Q],
                                )
                            pend.append((t, pr, pf))
                    while pend:
                        issue_pv(pend.popleft())

                    # ---- normalize, DVE part: compact numerators + 1/z ----
                    # (the PE-dependent broadcast matmul is deferred for mh0
                    # so it doesn't block mh1's QKs in the in-order PE queue)
                    if mh == 0:
                        # copy to SBUF, releasing oacc PSUM for mh1's PV
                        oaccS = [
                            npool.tile([KB, Q], f32, tag=f"oaccS{pr}",
                                       name=f"oaccS{pr}")
                            for pr in range(2)
                        ]
                        for pr in range(2):
                            nc.vector.tensor_copy(oaccS[pr][:], oacc[pr][:])
                        src = oaccS
                    else:
                        src = oacc  # tail: read PSUM directly
                    onum = npool.tile([KB, Q], f32, tag=f"onum{mh}",
                                      name=f"onum{mh}")
                    zsb = npool.tile([KB, Q], f32, tag="zsb", name="zsb")
                    nc.vector.memset(zsb[:], 1.0)
                    IDENT = mybir.ActivationFunctionType.Identity
                    for h4 in range(4):
                        pr, hp = h4 >> 1, h4 & 1
                        if mh == 1:
                            # tail: ACT is idle; run compaction there so it
                            # overlaps the DVE z-gather
                            nc.scalar.activation(
                                onum[32 * h4:32 * h4 + 32, :],
                                src[pr][64 * hp:64 * hp + 32, :], IDENT,
                            )
                        else:
                            nc.vector.tensor_copy(
                                onum[32 * h4:32 * h4 + 32, :],
                                src[pr][64 * hp:64 * hp + 32, :],
                            )
                        nc.vector.tensor_copy(
                            zsb[32 * h4:32 * h4 + 1, :],
                            src[pr][64 * hp + 32:64 * hp + 33, :],
                        )
                    rzb = npool.tile([KB, Q], bf16, tag=f"rzb{mh}",
                                     name=f"rzb{mh}")
                    if mh == 0:
                        # DVE reciprocal (ACT is saturated by the exp stream)
                        rz = npool.tile([KB, Q], f32, tag="rz", name="rz")
                        nc.vector.reciprocal_approx_fast(rz[:], zsb[:])
                        nc.vector.tensor_copy(rzb[:], rz[:])
                    else:
                        # tail: 1/z = exp(-ln z) on the now-idle ACT
                        lnz = npool.tile([KB, Q], f32, tag="lnz", name="lnz")
                        nc.scalar.activation(lnz[:], zsb[:], LN)
                        nc.scalar.activation(rzb[:], lnz[:], EXP, scale=-1.0)
                    norm_tail.append((mh, onum, rzb))

                # ---- normalize, PE part (after all attention matmuls) ----
                for mh, onum, rzb in norm_tail:
                    zbp = sps.tile([KB, Q], f32, tag="sg", name="zbp")
                    nc.tensor.matmul(
                        zbp[:], e128[:], rzb[:], start=True, stop=True
                    )
                    # mixed-space TT: exempt from equal-base-partition rule
                    nc.vector.tensor_mul(attnT[mh][:], onum[:], zbp[:])

            # ---- output projection + residual + LayerNorm ----
            with ExitStack() as ectx:
                rps = ectx.enter_context(
                    tc.tile_pool(name="rps", bufs=1, space="PSUM")
                )
                epool = ectx.enter_context(tc.tile_pool(name="epool", bufs=2))
                out2 = [
                    epool.tile([KB, Q], f32, tag=f"out2_{c}", name=f"out2_{c}")
                    for c in range(2)
                ]
                for c in range(2):
                    op_ps = rps.tile([KB, Q], f32, tag="oproj", name="oproj")
                    for mh in range(2):
                        nc.tensor.matmul(
                            op_ps[:],
                            wo[mh][:, c * KB:(c + 1) * KB],
                            attnT[mh][:],
                            start=(mh == 0), stop=(mh == 1),
                        )
                    # out2 = (psum + bo) + x
                    nc.vector.scalar_tensor_tensor(
                        out2[c][:], op_ps[:], bo[c][:], xq[c][:],
                        op0=ADD, op1=ADD,
                    )
                # stats: mu, s2 via ones matmuls (f32)
                ones_f = epool.tile([KB, 1], f32, tag="onesf", name="onesf")
                nc.vector.memset(ones_f[:], 1.0)
                mu_ps = rps.tile([1, Q], f32, tag="mu", name="mu")
                for c in range(2):
                    nc.tensor.matmul(
                        mu_ps[:], ones_f[:], out2[c][:],
                        start=(c == 0), stop=(c == 1),
                        skip_group_check=True,
                    )
                ones_b = epool.tile([KB, 1], bf16, tag="onesb", name="onesb")
                nc.vector.memset(ones_b[:], 1.0)
                s2_ps = rps.tile([1, Q], f32, tag="s2", name="s2")
                for c in range(2):
                    sq = epool.tile([KB, Q], bf16, tag="sq", name="sq")
                    nc.vector.tensor_mul(sq[:], out2[c][:], out2[c][:])
                    nc.tensor.matmul(
                        s2_ps[:], ones_b[:], sq[:],
                        start=(c == 0), stop=(c == 1),
                        skip_group_check=True,
                    )
                mu = epool.tile([1, Q], f32, tag="mu_s", name="mu_s")
                nc.vector.tensor_scalar_mul(mu[:], mu_ps[:], 1.0 / D)
                m2 = epool.tile([1, Q], f32, tag="m2", name="m2")
                nc.vector.tensor_mul(m2[:], mu[:], mu[:])
                var = epool.tile([1, Q], f32, tag="var", name="var")
                nc.vector.scalar_tensor_tensor(
                    var[:], s2_ps[:], 1.0 / D, m2[:], op0=MULT, op1=SUB,
                )
                # rstd = exp(-0.5*ln(var+eps)): stays in the exp/ln ACT
                # table set, avoiding a ~3us sqrt table switch
                lv = epool.tile([1, Q], f32, tag="lv", name="lv")
                nc.scalar.activation(lv[:], var[:], LN, bias=epsT[:])
                rstd = epool.tile([1, Q], f32, tag="rstd", name="rstd")
                nc.scalar.activation(rstd[:], lv[:], EXP, scale=-0.5)
                # broadcast tiles via outer products:
                # c1 = gamma (x) rstd ; c2 = beta (x) 1 - gamma (x) (mu*rstd)
                rstd_b = epool.tile([1, Q], bf16, tag="rstdb", name="rstdb")
                nc.vector.tensor_copy(rstd_b[:], rstd[:])
                mr = epool.tile([1, Q], f32, tag="mr", name="mr")
                nc.vector.tensor_mul(mr[:], mu[:], rstd[:])
                mrn = epool.tile([1, Q], bf16, tag="mrn", name="mrn")
                nc.vector.tensor_scalar_mul(mrn[:], mr[:], -1.0)
                for c in range(2):
                    csl = slice(c * KB, (c + 1) * KB)
                    c1p = rps.tile([KB, Q], f32, tag="c1", name="c1")
                    nc.tensor.matmul(
                        c1p[:], gm[:, csl], rstd_b[:], start=True, stop=True
                    )
                    # c2 = gamma (x) (-mu*rstd) + beta (x) 1
                    c2p = rps.tile([KB, Q], f32, tag="c2", name="c2")
                    nc.tensor.matmul(
                        c2p[:], gm[:, csl], mrn[:], start=True, stop=False
                    )
                    nc.tensor.matmul(
                        c2p[:], bt[:, csl], ones_1xQ[:],
                        start=False, stop=True,
                    )
                    t1 = epool.tile([KB, Q], f32, tag="t1", name="t1")
                    nc.vector.tensor_mul(t1[:], out2[c][:], c1p[:])
                    y = epool.tile([KB, Q], f32, tag="y", name="y")
                    nc.vector.tensor_add(y[:], t1[:], c2p[:])
                    nc.sync.dma_start(out=outT[csl, :], in_=y[:])

    nc.compile()
    return nc


def _prep_F(q_idx, k_idx, bias_eh):
    """Dense multiplicative bias F = exp(scattered bias), per core.

    Row-block order matches kernel consumption: [mh, t, h4, partition]."""
    key = q_idx.astype(np.int64) * N + k_idx.astype(np.int64)
    uk, inv = np.unique(key, return_inverse=True)
    acc = np.zeros((len(uk), H), np.float32)
    np.add.at(acc, inv, bias_eh)
    uq = (uk // N).astype(np.int32)
    ukey = (uk % N).astype(np.int32)
    vals16 = np.exp(acc).astype(ml_dtypes.bfloat16).view(np.uint16)

    Fs = []
    for i in range(NCORES):
        sel = (uq >> 9) == i
        q = uq[sel] & (Q - 1)
        k = ukey[sel]
        v = vals16[sel]
        t = k >> 8
        j = (k >> 7) & 1
        p = k & (KB - 1)
        # cols ordered (pr, j, hp, q) to match paired score tiles
        F16 = np.full((2, NPAIR, KB, 2, 2, 2, Q), 0x3F80, np.uint16)
        for h in range(H):
            F16[h >> 2, t, p, (h & 3) >> 1, j, h & 1, q] = v[:, h]
        Fs.append(
            np.ascontiguousarray(F16.reshape(2 * NPAIR * KB, 4 * 2 * Q))
            .view(ml_dtypes.bfloat16)
        )
    return Fs


def kernel(**inputs):
    global LAST_RESULTS, _PROG
    x = np.asarray(inputs["x"], np.float32)
    pos = np.asarray(inputs["pos_encoding"], np.float32)
    ei = np.asarray(inputs["edge_index"])
    et = np.asarray(inputs["edge_types"])
    emb = np.asarray(inputs["edge_emb"], np.float32)
    W = {k: np.asarray(inputs[k], np.float32) for k in ("Wq", "Wk", "Wv", "Wo")}
    b = {k: np.asarray(inputs[k], np.float32).reshape(-1)
         for k in ("bq", "bk", "bv", "bo", "gamma", "beta")}

    bias_eh = emb[et]  # [E, H]
    Fs = _prep_F(ei[0], ei[1], bias_eh)

    skip_bias = bool(np.all(b["bv"] == 0.0))
    pkey = (WARM_START, WARM_BOOST, WARM_PERIOD, WARM_LEN, skip_bias)
    if pkey not in _PROG:
        _PROG[pkey] = build_program(skip_bias=skip_bias)
    nc = _PROG[pkey]

    h = (x + pos).astype(np.float32)
    hT = np.ascontiguousarray(h.T.astype(ml_dtypes.bfloat16))
    # chunked layout (half, slab, part, 512)
    hTc = np.ascontiguousarray(
        hT.reshape(2, KB, 8, Q).transpose(0, 2, 1, 3).reshape(2 * 8 * KB, Q)
    )
    xT = np.ascontiguousarray(x.T)
    Wb = {k: np.ascontiguousarray(w.astype(ml_dtypes.bfloat16))
          for k, w in W.items()}
    col = lambda a: np.ascontiguousarray(a.reshape(D, 1))
    row16 = lambda a: np.ascontiguousarray(
        a.reshape(1, D).astype(ml_dtypes.bfloat16)
    )
    e128 = np.zeros((KB, KB), np.float32)
    for h4 in range(4):
        e128[32 * h4, 32 * h4:32 * h4 + 32] = 1.0
    e128 = np.ascontiguousarray(e128.astype(ml_dtypes.bfloat16))

    in_maps = []
    for i in range(NCORES):
        sl = slice(i * Q, (i + 1) * Q)
        in_maps.append({
            "hT": hTc,
            "hqT": np.ascontiguousarray(hT[:, sl]),
            "xqT": np.ascontiguousarray(xT[:, sl]),
            "Wq": Wb["Wq"], "Wk": Wb["Wk"], "Wv": Wb["Wv"], "Wo": Wb["Wo"],
            "bq": col(b["bq"]), "bk": col(b["bk"]), "bo": col(b["bo"]),
            "bv": row16(b["bv"]), "gm": row16(b["gamma"]),
            "bt": row16(b["beta"]), "e128": e128,
            "F": Fs[i],
        })

    trace = os.environ.get("BASS_KERNEL_TRACE", "0") == "1"
    try:
        res = run_bass_kernel_spmd(
            nc, in_maps, list(range(NCORES)), trace=trace
        )
    except Exception:
        if not trace:
            raise
        res = run_bass_kernel_spmd(nc, in_maps, list(range(NCORES)))
    LAST_RESULTS = res

    out = np.empty((N, D), np.float32)
    for i in range(NCORES):
        out[i * Q:(i + 1) * Q, :] = np.asarray(
            res.results[i]["outT"], np.float32
        ).T
    return out


# revision 74
# speedup vs baseline: 1.0233x; 1.0002x over previous
"""Graphormer layer (LocalSubgraphEncoder) Trainium2 Bass kernel, v2.

Sharding: node-parallel over 8 cores. Core i computes the full layer output
for query nodes [512*i, 512*i+512): all 8 heads of attention over all 4096
key nodes, edge-type bias, softmax, output projection, residual, LayerNorm.
No cross-core communication; host concatenates row slices.

v2 design (from perfetto analysis of v1: PE saturated by unpacked K=32
matmuls, GPSIMD dense local_scatter, STT stuck in 1x mode):
 - all matmuls bf16; 2-head row-packing for QK (tile_position row groups)
   and 2-head column-packing for PV / denominator matmuls.
 - scores layout S^T [keys(part), queries(free)]: softmax denominator z
   comes from a packed ones-vector matmul into a shared PSUM bank.
 - edge bias applied multiplicatively AFTER exp: P = exp(S) * F where
   F = exp(scattered bias) is precomputed DENSE on the host and streamed
   from HBM (33.5 MB/core) -> one 2x-mode DVE tensor_tensor per tile;
   GPSIMD does nothing.
 - ACT (ScalarE) does exclusively the exp drain PSUM->SBUF bf16 in
   [128,1024] tiles: the ~128 us floor every design shares.
 - biases fused into DVE copies (per-partition scalar AP) or rank-1 PE
   matmuls; LayerNorm scale/shift via outer-product matmuls.
"""
import os
import sys
import math
import numpy as np

sys.path.insert(0, "/opt/trn_rl_repo")
import ml_dtypes  # noqa: E402
from concourse import bacc, bass, mybir, tile  # noqa: E402
from concourse.bass_utils import run_bass_kernel_spmd  # noqa: E402

N, D, H, E, NT = 4096, 256, 8, 131072, 16
DH = D // H            # 32
NCORES = 8
Q = N // NCORES        # 512 query nodes per core
KB = 128               # key-node block (partition dim)
NKB = N // KB          # 32
NPAIR = NKB // 2       # 16 (two key-blocks per [128,1024] score tile)
LN_EPS = 1e-5
SCALE = 1.0 / math.sqrt(DH)

f32 = mybir.dt.float32
bf16 = mybir.dt.bfloat16
EXP = mybir.ActivationFunctionType.Exp
LN = mybir.ActivationFunctionType.Ln
ADD = mybir.AluOpType.add
MULT = mybir.AluOpType.mult
SUB = mybir.AluOpType.subtract

_PROG = {}
LAST_RESULTS = None

WARM_START = int(os.environ.get("WARM_START", "0"))
WARM_BOOST = int(os.environ.get("WARM_BOOST", "0"))
WARM_PERIOD = int(os.environ.get("WARM_PERIOD", "0"))
WARM_LEN = int(os.environ.get("WARM_LEN", "8"))
WARM_PROJ = int(os.environ.get("WARM_PROJ", "20"))


_TABLES_PATCHED = False


def _patch_act_tables():
    """Restrict the ACT table chooser to natural_log_exp_and_others (has
    exp, ln and identity) so the whole kernel needs ONE table load instead
    of bouncing between the exp and ln sets. Keys/order preserved so
    act_func_set_id indices stay valid."""
    global _TABLES_PATCHED
    if _TABLES_PATCHED:
        return
    from concourse import hw_specs
    import concourse.bacc as bacc_mod

    orig = hw_specs.get_activation_tables

    def patched(arch):
        t = orig(arch)
        keep = "natural_log_exp_and_others"
        if keep not in t:
            return t
        return {k: (v if k == keep else set()) for k, v in t.items()}

    bacc_mod.get_activation_tables = patched
    _TABLES_PATCHED = True


def build_program(skip_bias=False, skip_beta=False):
    _patch_act_tables()
    nc = bacc.Bacc(
        "TRN2", target_bir_lowering=False, debug=False, num_devices=NCORES
    )

    def din(name, shape, dt):
        return nc.dram_tensor(name, shape, dt, kind="ExternalInput").ap()

    # (x + pos)^T stored as (half, slab, part, 512) so each chunk is a
    # contiguous 128 KiB transfer and projections start on the first chunk
    hT_d = din("hT", [2 * 8 * KB, Q], bf16)
    xqT_d = din("xqT", [D, Q], f32)         # x^T core slice (residual)
    Wq_d = din("Wq", [D, D], bf16)
    Wk_d = din("Wk", [D, D], bf16)
    Wv_d = din("Wv", [D, D], bf16)
    Wo_d = din("Wo", [D, D], bf16)
    bq_d = din("bq", [D, 1], f32)
    bk_d = din("bk", [D, 1], f32)
    bo_d = din("bo", [D, 1], f32)
    bv_d = din("bv", [1, D], bf16)
    gm_d = din("gm", [1, D], bf16)          # gamma row
    bt_d = din("bt", [1, D], bf16)          # beta row
    e128_d = din("e128", [KB, KB], bf16)    # block-broadcast matrix
    # dense exp(bias): row = (mh, t, partition), col = (h4, j, q)
    F_d = din("F", [2 * NPAIR * KB, 4 * 2 * Q], bf16)
    outT = nc.dram_tensor("outT", [D, Q], f32, kind="ExternalOutput").ap()

    hqT_d = din("hqT", [D, Q], bf16)        # h^T core query slice

    with tile.TileContext(nc) as tc:
        from contextlib import ExitStack

        with ExitStack() as ctx:
            cpool = ctx.enter_context(tc.tile_pool(name="consts", bufs=1))

            def ctile(shape, dt, tag):
                return cpool.tile(shape, dt, tag=tag, name=tag)

            # persistent SBUF residents
            hT = [ctile([KB, N], bf16, f"hT{c}") for c in range(2)]
            hq = [ctile([KB, Q], bf16, f"hq{c}") for c in range(2)]
            xq = [ctile([KB, Q], f32, f"xq{c}") for c in range(2)]
            wq = [ctile([KB, D], bf16, f"wq{c}") for c in range(2)]
            wk = [ctile([KB, D], bf16, f"wk{c}") for c in range(2)]
            wv = [ctile([KB, D], bf16, f"wv{c}") for c in range(2)]
            wo = [ctile([KB, D], bf16, f"wo{c}") for c in range(2)]
            bq = [ctile([KB, 1], f32, f"bq{c}") for c in range(2)]
            bk = [ctile([KB, 1], f32, f"bk{c}") for c in range(2)]
            bo = [ctile([KB, 1], f32, f"bo{c}") for c in range(2)]
            bv_r = ctile([1, D], bf16, "bv_r")
            gm = ctile([1, D], bf16, "gm")
            bt = ctile([1, D], bf16, "bt")
            e128 = ctile([KB, KB], bf16, "e128")
            kT = [ctile([KB, N], bf16, f"kT{c}") for c in range(2)]
            qTb = [ctile([KB, Q], bf16, f"qTb{c}") for c in range(2)]
            # V with ones column: [key, kb, h, 32 dims + 1 one]
            vSB = ctile([KB, NKB, H, DH + 1], bf16, "vSB")
            attnT = [ctile([KB, Q], bf16, f"attnT{c}") for c in range(2)]
            ones_1x128 = ctile([1, KB], bf16, "o1x128")
            ones_128x1 = ctile([KB, 1], bf16, "o128x1")
            ones_1xQ = ctile([1, Q], bf16, "o1xQ")
            epsT = ctile([1, 1], f32, "epsT")
            zer_1xQ = ctile([1, Q], f32, "z1xQ")

            # ---- loads ----
            # DMA priority: Q-proj inputs first, then per-slab hT chunks
            # (both halves per slab so K-proj slab s starts ASAP)
            for c in range(2):
                sl = slice(c * KB, (c + 1) * KB)
                nc.sync.dma_start(out=wq[c][:], in_=Wq_d[sl, :])
                nc.sync.dma_start(out=hq[c][:], in_=hqT_d[sl, :])
                nc.sync.dma_start(out=wk[c][:], in_=Wk_d[sl, :])
                nc.sync.dma_start(out=wv[c][:], in_=Wv_d[sl, :])
                nc.sync.dma_start(out=bq[c][:], in_=bq_d[sl, :])
                nc.sync.dma_start(out=bk[c][:], in_=bk_d[sl, :])
            for s in range(8):
                for c in range(2):
                    row = (c * 8 + s) * KB
                    nc.sync.dma_start(
                        out=hT[c][:, s * Q:(s + 1) * Q],
                        in_=hT_d[row:row + KB, :],
                    )
            nc.sync.dma_start(out=bv_r[:], in_=bv_d[:])
            nc.sync.dma_start(out=e128[:], in_=e128_d[:])
            for c in range(2):
                sl = slice(c * KB, (c + 1) * KB)
                nc.sync.dma_start(out=wo[c][:], in_=Wo_d[sl, :])
                nc.sync.dma_start(out=xq[c][:], in_=xqT_d[sl, :])
                nc.sync.dma_start(out=bo[c][:], in_=bo_d[sl, :])
            nc.sync.dma_start(out=gm[:], in_=gm_d[:])
            nc.sync.dma_start(out=bt[:], in_=bt_d[:])
            nc.vector.memset(ones_1x128[:], 1.0)
            nc.vector.memset(ones_128x1[:], 1.0)
            nc.vector.memset(epsT[:], LN_EPS)
            nc.vector.memset(zer_1xQ[:], 0.0)

            # preload the exp ACT table during projections so the first real
            # exp doesn't stall the attention pipeline for ~2.7us; the output
            # is live (ones_1xQ = exp(0)) so DCE keeps it.
            nc.scalar.activation(ones_1xQ[:], zer_1xQ[:], EXP)

            # ---- HAM warm-up on memset data during the input DMA wait ----
            # (PE would otherwise idle ~10us for wq/hq; a dense dummy burst
            # flips the clock gate to 8/8 so projections run at 2.4 GHz)
            if WARM_PROJ:
                warm_w = ctile([KB, KB], bf16, "warm_w")
                warm_x = ctile([KB, Q], bf16, "warm_x")
                nc.vector.memset(warm_w[:], 0.0)
                nc.vector.memset(warm_x[:], 0.0)
                with tc.tile_pool(name="wps", bufs=1, space="PSUM") as wps:
                    wt = wps.tile([KB, Q], f32, tag="w", name="w")
                    for _ in range(WARM_PROJ):
                        nc.tensor.matmul(
                            wt[:], warm_w[:], warm_x[:],
                            start=True, stop=True,
                        )

            # ---- projections (all bf16, biases fused) ----
            with tc.tile_pool(name="pps", bufs=3, space="PSUM") as pps:
                IDENT = mybir.ActivationFunctionType.Identity
                # Q^T [2][128, 512] head-major partitions; bias fused into
                # the ACT copy (per-partition bias is native there)
                for mh in range(2):
                    ps = pps.tile([KB, Q], f32, tag="proj", name="proj")
                    for kc in range(2):
                        nc.tensor.matmul(
                            ps[:], wq[kc][:, mh * KB:(mh + 1) * KB], hq[kc][:],
                            start=(kc == 0), stop=(kc == 1),
                        )
                    nc.scalar.activation(
                        qTb[mh][:], ps[:], IDENT, bias=bq[mh][:]
                    )
                # K^T [2][128, 4096]  (interleave mh so attention t=0 for
                # both halves unblocks early)
                for s in range(8):
                    for mh in range(2):
                        ssl = slice(s * Q, (s + 1) * Q)
                        ps = pps.tile([KB, Q], f32, tag="proj", name="proj")
                        for kc in range(2):
                            nc.tensor.matmul(
                                ps[:], wk[kc][:, mh * KB:(mh + 1) * KB],
                                hT[kc][:, ssl],
                                start=(kc == 0), stop=(kc == 1),
                            )
                        nc.scalar.activation(
                            kT[mh][:, ssl], ps[:], IDENT, bias=bk[mh][:]
                        )
                # V node-major [128, kb, h, 33] + bias via rank-1
                nc.vector.memset(vSB[:, :, :, DH], 1.0)
                for kb_i in range(NKB):
                    ksl = slice(kb_i * KB, (kb_i + 1) * KB)
                    psv = pps.tile([KB, D], f32, tag="projv", name="projv")
                    for kc in range(2):
                        nc.tensor.matmul(
                            psv[:], hT[kc][:, ksl], wv[kc][:],
                            start=(kc == 0),
                            stop=(kc == 1 and skip_bias),
                        )
                    if not skip_bias:
                        nc.tensor.matmul(
                            psv[:], ones_1x128[:], bv_r[:],
                            start=False, stop=True,
                        )
                    nc.vector.tensor_copy(vSB[:, kb_i, :, 0:DH], psv[:])

            # ---- attention ----
            with ExitStack() as actx:
                sps = actx.enter_context(
                    tc.tile_pool(name="sps", bufs=3, space="PSUM")
                )
                ops = actx.enter_context(
                    tc.tile_pool(name="ops", bufs=2, space="PSUM")
                )
                spool = actx.enter_context(tc.tile_pool(name="spool", bufs=8))
                fpool = actx.enter_context(tc.tile_pool(name="fpool", bufs=3))
                npool = actx.enter_context(tc.tile_pool(name="npool", bufs=2))

                norm_tail = []
                for mh in range(2):
                    # one oacc tile per head pair: partitions 0:33 head A
                    # (32 dims + z), 64:97 head B.
                    oacc = [
                        ops.tile([KB, Q], f32, tag="oacc", name="oacc")
                        for _ in range(2)
                    ]

                    def issue_pv(g):
                        t, pr, pf = g
                        first = (t == 0)
                        last = (t == NPAIR - 1)
                        for j in range(2):
                            kb_i = 2 * t + j
                            for hp in range(2):
                                h4 = 2 * pr + hp
                                h = 4 * mh + h4
                                nc.tensor.matmul(
                                    oacc[pr][64 * hp:64 * hp + DH + 1, :],
                                    vSB[:, kb_i, h, :],
                                    pf[j][:, hp * Q:(hp + 1) * Q],
                                    start=(first and j == 0),
                                    stop=(last and j == 1),
                                    tile_position=(0, 64 * hp),
                                    skip_group_check=True,
                                )

                    # HAM warm-up: dependency-free burst of matmuls into the
                    # oacc region; the first real PV starts with start=True
                    # so the garbage is overwritten.
                    for w in range(WARM_START):
                        nc.tensor.matmul(
                            oacc[0][0:DH + 1, :], vSB[:, 0, 0, :], qTb[mh][:],
                            start=True, stop=True,
                            tile_position=(0, 0), skip_group_check=True,
                        )
                    # software pipeline, lag 2: PV(g-2) issues BEFORE QK(g)
                    # so the in-order PE always has dependency-free work.
                    from collections import deque
                    pend = deque()
                    for t in range(NPAIR):
                        if WARM_PERIOD and mh == 0 and t == WARM_PERIOD:
                            # periodic dense matmul burst to re-flip HAM
                            bt_ps = sps.tile([KB, 2 * Q], f32, tag="sg",
                                             name="warm")
                            for w in range(WARM_LEN):
                                nc.tensor.matmul(
                                    bt_ps[:, 0:Q],
                                    kT[mh][0:32, 0:KB], qTb[mh][0:32, :],
                                    start=True, stop=True,
                                    tile_position=(0, 0),
                                    skip_group_check=True,
                                )
                        # one 1 MiB F transfer covers (mh, t) x 4 heads
                        fbig = fpool.tile([KB, 8 * Q], bf16, tag="ft",
                                          name="ft")
                        row = (mh * NPAIR + t) * KB
                        nc.sync.dma_start(
                            out=fbig[:], in_=F_d[row:row + KB, :]
                        )
                        for pr in range(2):      # head pairs (2p, 2p+1)
                            if len(pend) >= 2:
                                issue_pv(pend.popleft())
                            # score tiles pair TWO HEADS at the same j so the
                            # two QK matmuls of a tile land on different PE
                            # row groups and run concurrently (row packing).
                            sg = [
                                sps.tile([KB, 2 * Q], f32, tag="sg", name="sg")
                                for _ in range(2)
                            ]
                            # density boosters (optional): dummy weight loads
                            for w in range(WARM_BOOST):
                                nc.tensor.ldweights(
                                    kT[mh][:, 0:KB], tile_position=(0, 0),
                                )
                            for j in range(2):
                                kb_i = 2 * t + j
                                ksl = slice(kb_i * KB, (kb_i + 1) * KB)
                                for hp in range(2):
                                    h4 = 2 * pr + hp
                                    psl = slice(32 * h4, 32 * h4 + 32)
                                    nc.tensor.matmul(
                                        sg[j][:, hp * Q:(hp + 1) * Q],
                                        kT[mh][psl, ksl],
                                        qTb[mh][psl, :],
                                        start=True, stop=True,
                                        tile_position=(32 * h4, 0),
                                    )
                            pf = [None, None]
                            for j in range(2):
                                # exp (ACT) PSUM -> SBUF bf16
                                p0 = spool.tile(
                                    [KB, 2 * Q], bf16, tag="p0", name="p0"
                                )
                                nc.scalar.activation(
                                    p0[:], sg[j][:], EXP, scale=SCALE
                                )
                                pf[j] = spool.tile(
                                    [KB, 2 * Q], bf16, tag="pf", name="pf"
                                )
                                nc.vector.tensor_mul(
                                    pf[j][:], p0[:],
                                    fbig[:, (pr * 2 + j) * 2 * Q:
                                         (pr * 2 + j + 1) * 2 * Q],
                                )
                            pend.append((t, pr, pf))
                    while pend:
                        issue_pv(pend.popleft())

                    # ---- normalize, DVE part: compact numerators + 1/z ----
                    # (the PE-dependent broadcast matmul is deferred for mh0
                    # so it doesn't block mh1's QKs in the in-order PE queue)
                    if mh == 0:
                        # copy to SBUF, releasing oacc PSUM for mh1's PV
                        oaccS = [
                            npool.tile([KB, Q], f32, tag=f"oaccS{pr}",
                                       name=f"oaccS{pr}")
                            for pr in range(2)
                        ]
                        for pr in range(2):
                            nc.vector.tensor_copy(oaccS[pr][:], oacc[pr][:])
                        src = oaccS
                    else:
                        src = oacc  # tail: read PSUM directly
                    onum = npool.tile([KB, Q], f32, tag=f"onum{mh}",
                                      name=f"onum{mh}")
                    zsb = npool.tile([KB, Q], f32, tag="zsb", name="zsb")
                    nc.vector.memset(zsb[:], 1.0)
                    IDENT = mybir.ActivationFunctionType.Identity
                    for h4 in range(4):
                        pr, hp = h4 >> 1, h4 & 1
                        if mh == 1:
                            # tail: ACT is idle; run compaction there so it
                            # overlaps the DVE z-gather
                            nc.scalar.activation(
                                onum[32 * h4:32 * h4 + 32, :],
                                src[pr][64 * hp:64 * hp + 32, :], IDENT,
                            )
                        else:
                            nc.vector.tensor_copy(
                                onum[32 * h4:32 * h4 + 32, :],
                                src[pr][64 * hp:64 * hp + 32, :],
                            )
                        nc.vector.tensor_copy(
                            zsb[32 * h4:32 * h4 + 1, :],
                            src[pr][64 * hp + 32:64 * hp + 33, :],
                        )
                    rzb = npool.tile([KB, Q], bf16, tag=f"rzb{mh}",
                                     name=f"rzb{mh}")
                    if mh == 0:
                        # DVE reciprocal (ACT is saturated by the exp stream)
                        rz = npool.tile([KB, Q], f32, tag="rz", name="rz")
                        nc.vector.reciprocal_approx_fast(rz[:], zsb[:])
                        nc.vector.tensor_copy(rzb[:], rz[:])
                    else:
                        # tail: 1/z = exp(-ln z) on the now-idle ACT
                        lnz = npool.tile([KB, Q], f32, tag="lnz", name="lnz")
                        nc.scalar.activation(lnz[:], zsb[:], LN)
                        nc.scalar.activation(rzb[:], lnz[:], EXP, scale=-1.0)
                    norm_tail.append((mh, onum, rzb))

                # ---- normalize, PE part (after all attention matmuls) ----
                for mh, onum, rzb in norm_tail:
                    zbp = sps.tile([KB, Q], f32, tag="sg", name="zbp")
                    nc.tensor.matmul(
                        zbp[:], e128[:], rzb[:], start=True, stop=True
                    )
                    # mixed-space TT: exempt from equal-base-partition rule
                    nc.vector.tensor_mul(attnT[mh][:], onum[:], zbp[:])

            # ---- output projection + residual + LayerNorm ----
            with ExitStack() as ectx:
                rps = ectx.enter_context(
                    tc.tile_pool(name="rps", bufs=1, space="PSUM")
                )
                epool = ectx.enter_context(tc.tile_pool(name="epool", bufs=2))
                out2 = [
                    epool.tile([KB, Q], f32, tag=f"out2_{c}", name=f"out2_{c}")
                    for c in range(2)
                ]
                for c in range(2):
                    op_ps = rps.tile([KB, Q], f32, tag="oproj", name="oproj")
                    for mh in range(2):
                        nc.tensor.matmul(
                            op_ps[:],
                            wo[mh][:, c * KB:(c + 1) * KB],
                            attnT[mh][:],
                            start=(mh == 0), stop=(mh == 1),
                        )
                    # out2 = (psum + bo) + x
                    nc.vector.scalar_tensor_tensor(
                        out2[c][:], op_ps[:], bo[c][:], xq[c][:],
                        op0=ADD, op1=ADD,
                    )
                # stats: mu, s2 via ones matmuls (f32)
                ones_f = epool.tile([KB, 1], f32, tag="onesf", name="onesf")
                nc.vector.memset(ones_f[:], 1.0)
                mu_ps = rps.tile([1, Q], f32, tag="mu", name="mu")
                for c in range(2):
                    nc.tensor.matmul(
                        mu_ps[:], ones_f[:], out2[c][:],
                        start=(c == 0), stop=(c == 1),
                        skip_group_check=True,
                    )
                ones_b = epool.tile([KB, 1], bf16, tag="onesb", name="onesb")
                nc.vector.memset(ones_b[:], 1.0)
                s2_ps = rps.tile([1, Q], f32, tag="s2", name="s2")
                for c in range(2):
                    sq = epool.tile([KB, Q], bf16, tag="sq", name="sq")
                    # Square on ACT (same table set) — overlaps the DVE chain
                    nc.scalar.activation(
                        sq[:], out2[c][:],
                        mybir.ActivationFunctionType.Square,
                    )
                    nc.tensor.matmul(
                        s2_ps[:], ones_b[:], sq[:],
                        start=(c == 0), stop=(c == 1),
                        skip_group_check=True,
                    )
                mu = epool.tile([1, Q], f32, tag="mu_s", name="mu_s")
                nc.vector.tensor_scalar_mul(mu[:], mu_ps[:], 1.0 / D)
                m2 = epool.tile([1, Q], f32, tag="m2", name="m2")
                nc.vector.tensor_mul(m2[:], mu[:], mu[:])
                var = epool.tile([1, Q], f32, tag="var", name="var")
                nc.vector.scalar_tensor_tensor(
                    var[:], s2_ps[:], 1.0 / D, m2[:], op0=MULT, op1=SUB,
                )
                # rstd = exp(-0.5*ln(var+eps)): stays in the exp/ln ACT
                # table set, avoiding a ~3us sqrt table switch
                lv = epool.tile([1, Q], f32, tag="lv", name="lv")
                nc.scalar.activation(lv[:], var[:], LN, bias=epsT[:])
                rstd = epool.tile([1, Q], f32, tag="rstd", name="rstd")
                nc.scalar.activation(rstd[:], lv[:], EXP, scale=-0.5)
                # broadcast tiles via outer products:
                # c1 = gamma (x) rstd ; c2 = beta (x) 1 - gamma (x) (mu*rstd)
                rstd_b = epool.tile([1, Q], bf16, tag="rstdb", name="rstdb")
                nc.vector.tensor_copy(rstd_b[:], rstd[:])
                mr = epool.tile([1, Q], f32, tag="mr", name="mr")
                nc.vector.tensor_mul(mr[:], mu[:], rstd[:])
                mrn = epool.tile([1, Q], bf16, tag="mrn", name="mrn")
                nc.vector.tensor_scalar_mul(mrn[:], mr[:], -1.0)
                for c in range(2):
                    csl = slice(c * KB, (c + 1) * KB)
                    c1p = rps.tile([KB, Q], f32, tag="c1", name="c1")
                    nc.tensor.matmul(
                        c1p[:], gm[:, csl], rstd_b[:], start=True, stop=True
                    )
                    # c2 = gamma (x) (-mu*rstd) + beta (x) 1
                    c2p = rps.tile([KB, Q], f32, tag="c2", name="c2")
                    nc.tensor.matmul(
                        c2p[:], gm[:, csl], mrn[:],
                        start=True, stop=skip_beta,
                    )
                    if not skip_beta:
                        nc.tensor.matmul(
                            c2p[:], bt[:, csl], ones_1xQ[:],
                            start=False, stop=True,
                        )
                    t1 = epool.tile([KB, Q], f32, tag="t1", name="t1")
                    nc.vector.tensor_mul(t1[:], out2[c][:], c1p[:])
                    y = epool.tile([KB, Q], f32, tag="y", name="y")
                    nc.vector.tensor_add(y[:], t1[:], c2p[:])
                    nc.sync.dma_start(out=outT[csl, :], in_=y[:])

    nc.compile()
    return nc


def _prep_F(q_idx, k_idx, bias_eh):
    """Dense multiplicative bias F = exp(scattered bias), per core.

    Row-block order matches kernel consumption: [mh, t, h4, partition]."""
    key = q_idx.astype(np.int64) * N + k_idx.astype(np.int64)
    uk, inv = np.unique(key, return_inverse=True)
    acc = np.zeros((len(uk), H), np.float32)
    np.add.at(acc, inv, bias_eh)
    uq = (uk // N).astype(np.int32)
    ukey = (uk % N).astype(np.int32)
    vals16 = np.exp(acc).astype(ml_dtypes.bfloat16).view(np.uint16)

    Fs = []
    for i in range(NCORES):
        sel = (uq >> 9) == i
        q = uq[sel] & (Q - 1)
        k = ukey[sel]
        v = vals16[sel]
        t = k >> 8
        j = (k >> 7) & 1
        p = k & (KB - 1)
        # cols ordered (pr, j, hp, q) to match paired score tiles
        F16 = np.full((2, NPAIR, KB, 2, 2, 2, Q), 0x3F80, np.uint16)
        for h in range(H):
            F16[h >> 2, t, p, (h & 3) >> 1, j, h & 1, q] = v[:, h]
        Fs.append(
            np.ascontiguousarray(F16.reshape(2 * NPAIR * KB, 4 * 2 * Q))
            .view(ml_dtypes.bfloat16)
        )
    return Fs


def kernel(**inputs):
    global LAST_RESULTS, _PROG
    x = np.asarray(inputs["x"], np.float32)
    pos = np.asarray(inputs["pos_encoding"], np.float32)
    ei = np.asarray(inputs["edge_index"])
    et = np.asarray(inputs["edge_types"])
    emb = np.asarray(inputs["edge_emb"], np.float32)
    W = {k: np.asarray(inputs[k], np.float32) for k in ("Wq", "Wk", "Wv", "Wo")}
    b = {k: np.asarray(inputs[k], np.float32).reshape(-1)
         for k in ("bq", "bk", "bv", "bo", "gamma", "beta")}

    bias_eh = emb[et]  # [E, H]
    Fs = _prep_F(ei[0], ei[1], bias_eh)

    skip_bias = bool(np.all(b["bv"] == 0.0))
    skip_beta = bool(np.all(b["beta"] == 0.0))
    pkey = (WARM_START, WARM_BOOST, WARM_PERIOD, WARM_LEN, WARM_PROJ,
            skip_bias, skip_beta)
    if pkey not in _PROG:
        _PROG[pkey] = build_program(skip_bias=skip_bias, skip_beta=skip_beta)
    nc = _PROG[pkey]

    h = (x + pos).astype(np.float32)
    hT = np.ascontiguousarray(h.T.astype(ml_dtypes.bfloat16))
    # chunked layout (half, slab, part, 512)
    hTc = np.ascontiguousarray(
        hT.reshape(2, KB, 8, Q).transpose(0, 2, 1, 3).reshape(2 * 8 * KB, Q)
    )
    xT = np.ascontiguousarray(x.T)
    Wb = {k: np.ascontiguousarray(w.astype(ml_dtypes.bfloat16))
          for k, w in W.items()}
    col = lambda a: np.ascontiguousarray(a.reshape(D, 1))
    row16 = lambda a: np.ascontiguousarray(
        a.reshape(1, D).astype(ml_dtypes.bfloat16)
    )
    e128 = np.zeros((KB, KB), np.float32)
    for h4 in range(4):
        e128[32 * h4, 32 * h4:32 * h4 + 32] = 1.0
    e128 = np.ascontiguousarray(e128.astype(ml_dtypes.bfloat16))

    in_maps = []
    for i in range(NCORES):
        sl = slice(i * Q, (i + 1) * Q)
        in_maps.append({
            "hT": hTc,
            "hqT": np.ascontiguousarray(hT[:, sl]),
            "xqT": np.ascontiguousarray(xT[:, sl]),
            "Wq": Wb["Wq"], "Wk": Wb["Wk"], "Wv": Wb["Wv"], "Wo": Wb["Wo"],
            "bq": col(b["bq"]), "bk": col(b["bk"]), "bo": col(b["bo"]),
            "bv": row16(b["bv"]), "gm": row16(b["gamma"]),
            "bt": row16(b["beta"]), "e128": e128,
            "F": Fs[i],
        })

    trace = os.environ.get("BASS_KERNEL_TRACE", "0") == "1"
    try:
        res = run_bass_kernel_spmd(
            nc, in_maps, list(range(NCORES)), trace=trace
        )
    except Exception:
        if not trace:
            raise
        res = run_bass_kernel_spmd(nc, in_maps, list(range(NCORES)))
    LAST_RESULTS = res

    out = np.empty((N, D), np.float32)
    for i in range(NCORES):
        out[i * Q:(i + 1) * Q, :] = np.asarray(
            res.results[i]["outT"], np.float32
        ).T
    return out
